# revision 8
# baseline (speedup 1.0000x reference)
"""Trainium2 Bass kernel: memory-slot cross-attention (nn_LocalConstructorMulti).

Reference computation (per batch b):
    Q  = memory_slots @ Wq.T                      [slots, BD]    (shared over b)
    K  = hs_b @ Wk.T                              [S, BD]
    V  = hs_b @ Wv.T                              [S, BD]
    s  = (Q_h . K_h) / sqrt(HD)  + mask           [heads, slots, S]
    p  = softmax(s, axis=S)
    o  = p @ V_h                                  [heads, slots, HD]
    y  = concat_h(o) @ Wo.T                       [slots, HID]

Sharding: 8 cores = 4 batches x 2 head-groups (4 heads / 256 bottleneck dims
each).  Each core sees the full hidden states of its batch and a 256-wide
slice of Wk/Wv/Wo, computes the full softmax locally over its heads, and
produces a partial y (contribution of its 4 heads).  The host sums the two
partials per batch -- o_proj is linear in the per-head outputs.

Key layout/engine decisions (v2, rebuilt from the TimelineSim bottleneck
analysis of v1: PE sequencer + HWDGE were saturated by 2434 small matmuls and
519 small-descriptor DMAs while the PE array itself was 45% idle):

  - hs is shipped twice, host-preshuffled into [8 blocks, 128 ki, 16 pair,
    2 two, 512 col] so each block is ONE DMA with 16-32 KiB contiguous
    per-partition lines: an fp8 copy (x8 scale) feeding the K-projection and
    a bf16 copy feeding the V-projection.  19 DMAs total vs 519.
  - K-projection runs in fp8 with MatmulPerfMode.DoubleRow: stationary
    wk8[128,2,128] x moving hs8[128,2,512] contracts TWO 128-deep k-subtiles
    per instruction (256 instructions for the whole KT).  fp8 noise on the
    K side is attenuated ~64x by the near-uniform softmax, contributing
    ~0.1% to the output.  Wk is pre-scaled x64 so its N(0, 1/4096) entries
    sit in e4m3's normal range; the combined 512x scale is folded into Q.
  - V-projection stays bf16 (V noise passes straight into the output):
    stationary hs blocks [128,128] x moving wv [128,256], PSUM-accumulated
    over all 32 k-subtiles, 1024 instructions at full 128x128x256 occupancy.
  - scores are built per 128-row tile with TWO head-pairs stacked on
    partitions: stationary kt[128, 128rows] x moving block-diagonal
    q2[128, 16] -> 64 matmuls; mask is a per-partition bias fused into Exp.
  - o = p^T @ V_aug runs as ONE 32-instruction PSUM chain: stationary
    pt[128, 4h*8n] x moving v[128, 4h*65] computes all 4 heads at once
    (the off-diagonal head blocks are discarded); the ones-column yields
    sum(p) for free, normalization is a per-partition scalar multiply.
  - o_proj is emitted transposed: yT[8, 4096] = ot[bd,8]^T @ woT[bd,4096] in
    16 mov-512 matmuls; the host adds the two head-group partials per batch.
  - Q (8x512, 0.02% of FLOPs) is computed on host and shipped pre-scaled as
    the block-diagonal q2 operand; every PSUM accumulator owns a full bank
    (PE-W + DVE/ACT-R same-bank erratum).
"""

import sys

if "/opt/trn_rl_repo" not in sys.path:
    sys.path.insert(0, "/opt/trn_rl_repo")

import ml_dtypes
import numpy as np

import concourse.bass as bass  # noqa: F401  (AP helpers)
import concourse.mybir as mybir
import concourse.tile as tile
from concourse import bacc
from concourse.bass_utils import run_bass_kernel_spmd
from concourse.masks import make_identity

BF16 = mybir.dt.bfloat16
FP8 = mybir.dt.float8e4
F32 = mybir.dt.float32
npbf16 = ml_dtypes.bfloat16
npfp8 = ml_dtypes.float8_e4m3

B, S, HID = 4, 4096, 4096
SLOTS, HEADS, BD = 8, 8, 512
HD = BD // HEADS  # 64
N_CORES = 8
GROUPS = N_CORES // B  # head-groups per batch
HPC = HEADS // GROUPS  # heads per core = 4
BDC = HPC * HD  # bottleneck slice per core = 256
MASK_NEG = -30000.0

HS_SCALE = 8.0  # fp8 hs pre-scale (lifts N(0,1) out of e4m3 subnormals)
WK_SCALE = 64.0  # fp8 Wk pre-scale (Wk entries ~ N(0, 1/4096))
# total score scale 1/sqrt(HD) divided back out of the device-side K product
Q_SCALE = 1.0 / (np.sqrt(HD) * HS_SCALE * WK_SCALE)

NBLK = 8  # column blocks of the sequence
CBLK = S // NBLK  # 512 columns per block
NPAIR = HID // 256  # 16 k-subtile pairs
NRT = S // 128  # 32 row tiles

# test.py can flip this to capture an NTFF profile; harness never touches it.
TRACE = False
TRACE_CORES = None
LAST_RESULT = None

_cache = {}


def _build_module():
    """Emit + compile the single-core Bass module (same NEFF on all cores)."""
    nc = bacc.Bacc("TRN2", target_bir_lowering=False, debug=False, num_devices=N_CORES)

    hs8T = nc.dram_tensor("hs8T", [NBLK, 128, NPAIR, 2, CBLK], FP8, kind="ExternalInput").ap()
    hsbT = nc.dram_tensor("hsbT", [NBLK, 128, NPAIR, 2, CBLK], BF16, kind="ExternalInput").ap()
    wk8T = nc.dram_tensor("wk8T", [128, NPAIR, 2, BDC], FP8, kind="ExternalInput").ap()
    wvT = nc.dram_tensor("wvT", [128, NPAIR, 2, BDC], BF16, kind="ExternalInput").ap()
    woT = nc.dram_tensor("woT", [128, 2, HID], BF16, kind="ExternalInput").ap()
    q2T = nc.dram_tensor("q2T", [128, 2, 2 * SLOTS], BF16, kind="ExternalInput").ap()
    mbT = nc.dram_tensor("mbT", [128, NRT], F32, kind="ExternalInput").ap()
    ypT = nc.dram_tensor("ypT", [SLOTS, HID], F32, kind="ExternalOutput").ap()

    DR = mybir.MatmulPerfMode.DoubleRow

    with tile.TileContext(nc) as tc:
        with (
            tc.tile_pool(name="consts", bufs=1) as consts,
            tc.tile_pool(name="hs8p", bufs=2) as hs8p,
            tc.tile_pool(name="hsbp", bufs=2) as hsbp,
        ):
            # ---- resident weights / tables -------------------------------
            wk8_sb = consts.tile([128, NPAIR, 2, BDC], FP8)
            nc.sync.dma_start(out=wk8_sb, in_=wk8T)
            wv_sb = consts.tile([128, NPAIR, 2, BDC], BF16)
            nc.sync.dma_start(out=wv_sb, in_=wvT)
            # wo/q2/mb are DMA'd after block 0's hs tiles (they're needed
            # only by the lagged attention / final o_proj)
            wo_sb = consts.tile([128, 2, HID], BF16)
            q2_sb = consts.tile([128, 2, 2 * SLOTS], BF16)
            mb_sb = consts.tile([128, NRT], F32)
            ident = consts.tile([128, 128], BF16)
            make_identity(nc, ident)

            # ---- persistent intermediates --------------------------------
            kt_sb = consts.tile([128, 2, S], BF16)  # K.T x512  [bd, rows]
            v_sb = consts.tile([128, NRT, HPC, HD + 1], BF16)  # V rows + ones
            nc.vector.memset(v_sb[:, :, :, HD : HD + 1], 1.0)
            # exp(scores).T, head stride padded to 32 so each head's o-block
            # lands on a 32-partition boundary (engine partition-offset rule)
            pt_sb = consts.tile([128, NRT, HPC, 32], BF16)
            nc.vector.memset(pt_sb, 0.0)

            # ---- K/V projections + lagged attention, one streamed pass ---
            # attention for block b-1 (scores -> exp -> o-chain partial) is
            # emitted after block b's projections, so the PE never waits on
            # the ACT exp round-trip and the old 15-20us serial tail folds
            # into the DMA/PE overlap window.
            RPB = CBLK // 128  # rowtiles per block

            def emit_attn(blk):
                for i in range(blk * RPB, (blk + 1) * RPB):
                    s_ps = sps.tile([128, 512], F32, tag="s")
                    for m2 in range(2):
                        nc.tensor.matmul(
                            s_ps[:, m2 * 16 : (m2 + 1) * 16],
                            kt_sb[:, m2, i * 128 : (i + 1) * 128],
                            q2_sb[:, m2, :],
                            start=True,
                            stop=True,
                        )
                    nc.scalar.activation(
                        out=pt_sb[:, i, :, 0:SLOTS],
                        in_=s_ps[:, 0 : HPC * SLOTS].rearrange(
                            "p (h n) -> p h n", h=HPC
                        ),
                        func=mybir.ActivationFunctionType.Exp,
                        bias=mb_sb[:, i : i + 1],
                        scale=1.0,
                    )
                for i in range(blk * RPB, (blk + 1) * RPB):
                    nc.tensor.matmul(
                        o_ps[:, 0 : HPC * (HD + 1)],
                        pt_sb[:, i, :, :],
                        v_sb[:, i, :, :],
                        start=(i == 0),
                        stop=(i == NRT - 1),
                    )

            oc = consts.tile([SLOTS, HPC, HD + 1], F32)
            with (
                tc.tile_pool(name="ktps", bufs=2, space="PSUM") as ktps,
                tc.tile_pool(name="vps", bufs=2, space="PSUM") as vps,
                tc.tile_pool(name="sps", bufs=2, space="PSUM") as sps,
                tc.tile_pool(name="ops", bufs=1, space="PSUM") as ops,
            ):
                o_ps = ops.tile([128, 512], F32)
                for blk in range(NBLK):
                    hs8_t = hs8p.tile([128, NPAIR, 2, CBLK], FP8, tag="hs8")
                    nc.sync.dma_start(out=hs8_t, in_=hs8T[blk])
                    hsb_t = hsbp.tile([128, NPAIR, 2, CBLK], BF16, tag="hsb")
                    nc.sync.dma_start(out=hsb_t, in_=hsbT[blk])
                    if blk == 0:
                        nc.sync.dma_start(out=q2_sb, in_=q2T)
                        nc.sync.dma_start(out=mb_sb, in_=mbT)
                        nc.sync.dma_start(out=wo_sb, in_=woT)
                    # KT chains: fp8 DoubleRow, 2 k-subtiles per instruction
                    for m2 in range(2):
                        kt_ps = ktps.tile([128, CBLK], F32, tag="kt")
                        for p in range(NPAIR):
                            nc.tensor.matmul(
                                kt_ps,
                                wk8_sb[:, p, :, m2 * 128 : (m2 + 1) * 128],
                                hs8_t[:, p, :, :],
                                start=(p == 0),
                                stop=(p == NPAIR - 1),
                                perf_mode=DR,
                            )
                        nc.scalar.copy(
                            out=kt_sb[:, m2, blk * CBLK : (blk + 1) * CBLK],
                            in_=kt_ps,
                        )
                    # V chains: bf16, stationary hs blocks, full-bank psum
                    for r in range(RPB):
                        v_ps = vps.tile([128, 512], F32, tag="v")
                        for p in range(NPAIR):
                            for t in range(2):
                                nc.tensor.matmul(
                                    v_ps[:, 0:BDC],
                                    hsb_t[:, p, t, r * 128 : (r + 1) * 128],
                                    wv_sb[:, p, t, :],
                                    start=(p == 0 and t == 0),
                                    stop=(p == NPAIR - 1 and t == 1),
                                )
                        rt = blk * RPB + r
                        nc.vector.tensor_copy(
                            out=v_sb[:, rt, :, 0:HD],
                            in_=v_ps[:, 0:BDC].rearrange("p (h d) -> p h d", h=HPC),
                        )
                    if blk > 0:
                        emit_attn(blk - 1)
                emit_attn(NBLK - 1)
                for h in range(HPC):
                    nc.scalar.copy(
                        out=oc[:, h, :],
                        in_=o_ps[h * 32 : h * 32 + SLOTS,
                                 h * (HD + 1) : (h + 1) * (HD + 1)],
                    )

            # normalize: o / sum(p) via the ones column
            recip = consts.tile([SLOTS, HPC], F32)
            o_slot = consts.tile([SLOTS, BDC], BF16)
            for h in range(HPC):
                nc.vector.reciprocal(
                    out=recip[:, h : h + 1], in_=oc[:, h, HD : HD + 1]
                )
                nc.vector.tensor_scalar_mul(
                    out=o_slot[:, h * HD : (h + 1) * HD],
                    in0=oc[:, h, 0:HD],
                    scalar1=recip[:, h : h + 1],
                )

            # ---- transpose o to [bd, slots] ------------------------------
            ot_sb = consts.tile([128, 2, SLOTS], BF16)
            with tc.tile_pool(name="tps", bufs=2, space="PSUM") as tps:
                for j in range(2):
                    t_ps = tps.tile([128, 1024], BF16, tag="t")
                    nc.tensor.transpose(
                        t_ps[:, 0:SLOTS],
                        o_slot[:, j * 128 : (j + 1) * 128],
                        ident[:SLOTS, :SLOTS],
                    )
                    nc.scalar.copy(out=ot_sb[:, j, :], in_=t_ps[:, 0:SLOTS])

            # ---- partial o_proj, transposed: yT = ot^T @ woT -------------
            y_sb = consts.tile([SLOTS, HID], F32)
            with tc.tile_pool(name="yps", bufs=2, space="PSUM") as yps:
                for seg in range(HID // 512):
                    y_ps = yps.tile([SLOTS, 512], F32, tag="y")
                    for j in range(2):
                        nc.tensor.matmul(
                            y_ps,
                            ot_sb[:, j, :],
                            wo_sb[:, j, seg * 512 : (seg + 1) * 512],
                            start=(j == 0),
                            stop=(j == 1),
                        )
                    nc.scalar.copy(
                        out=y_sb[:, seg * 512 : (seg + 1) * 512], in_=y_ps
                    )
                nc.sync.dma_start(out=ypT, in_=y_sb)

    nc.compile()
    return nc


def _get_module():
    if "m" not in _cache:
        _cache["m"] = _build_module()
    return _cache["m"]


def _shuffle_hs(hsT_np, dtype, scale=1.0):
    """[HID, S] -> [NBLK, 128, NPAIR, 2, CBLK] with the (pair, two, ki)
    k-decomposition on axis 0 and (blk, col) on axis 1."""
    a = hsT_np.reshape(NPAIR, 2, 128, NBLK, CBLK)
    a = a.transpose(3, 2, 0, 1, 4)  # blk, ki, pair, two, col
    if scale != 1.0:
        a = a * np.float32(scale)
    return np.ascontiguousarray(a.astype(dtype))


def _prep_in_maps(hs, mask, ms, Wq, Wk, Wv, Wo):
    """Shard the full inputs into 8 per-core input maps (host-side)."""
    WkT = Wk.T.astype(np.float32)  # [HID, BD]
    WvT = Wv.T.astype(np.float32)
    WoT = Wo.T.astype(np.float32)  # [BD, HID]
    Q = (ms @ Wq.T).astype(np.float32)  # [SLOTS, BD]

    hs8 = []
    hsb = []
    mb = []
    for b in range(B):
        hsT = np.ascontiguousarray(hs[b].T)  # [HID, S]
        hs8.append(_shuffle_hs(hsT, npfp8, HS_SCALE))
        hsb.append(_shuffle_hs(hsT, npbf16))
        mb.append(
            np.ascontiguousarray(
                np.where(mask[b] == 0, np.float32(MASK_NEG), np.float32(0.0))
                .astype(np.float32)
                .reshape(NRT, 128)
                .T
            )
        )

    in_maps = []
    for c in range(N_CORES):
        b, g = c // GROUPS, c % GROUPS
        sl = slice(g * BDC, (g + 1) * BDC)
        wk8 = (
            (WkT[:, sl] * np.float32(WK_SCALE))
            .reshape(NPAIR, 2, 128, BDC)
            .transpose(2, 0, 1, 3)
        )
        wv = WvT[:, sl].reshape(NPAIR, 2, 128, BDC).transpose(2, 0, 1, 3)
        wo = WoT[sl].reshape(2, 128, HID).transpose(1, 0, 2)
        q2 = np.zeros((128, 2, 2 * SLOTS), np.float32)
        for m2 in range(2):
            h0 = g * HPC + 2 * m2
            q2[0:64, m2, 0:SLOTS] = Q[:, h0 * HD : (h0 + 1) * HD].T * Q_SCALE
            q2[64:128, m2, SLOTS : 2 * SLOTS] = (
                Q[:, (h0 + 1) * HD : (h0 + 2) * HD].T * Q_SCALE
            )
        in_maps.append(
            {
                "hs8T": hs8[b],
                "hsbT": hsb[b],
                "wk8T": np.ascontiguousarray(wk8.astype(npfp8)),
                "wvT": np.ascontiguousarray(wv.astype(npbf16)),
                "woT": np.ascontiguousarray(wo.astype(npbf16)),
                "q2T": np.ascontiguousarray(q2.astype(npbf16)),
                "mbT": mb[b],
            }
        )
    return in_maps


def kernel(hidden_states, attention_mask, memory_slots, Wq, Wk, Wv, Wo):
    global LAST_RESULT
    hs = np.asarray(hidden_states, dtype=np.float32)
    mask = np.asarray(attention_mask)
    ms = np.asarray(memory_slots, dtype=np.float32)
    Wq = np.asarray(Wq, dtype=np.float32)
    Wk = np.asarray(Wk, dtype=np.float32)
    Wv = np.asarray(Wv, dtype=np.float32)
    Wo = np.asarray(Wo, dtype=np.float32)

    nc = _get_module()
    in_maps = _prep_in_maps(hs, mask, ms, Wq, Wk, Wv, Wo)

    kwargs = {}
    if TRACE:
        kwargs = {"trace": True}
        if TRACE_CORES is not None:
            kwargs["trace_cores"] = TRACE_CORES
    res = run_bass_kernel_spmd(nc, in_maps, core_ids=list(range(N_CORES)), **kwargs)
    LAST_RESULT = res

    yp = [r["ypT"] for r in res.results]  # each [SLOTS, HID] f32
    y = np.stack(
        [yp[GROUPS * b] + yp[GROUPS * b + 1] for b in range(B)], axis=0
    )
    return np.ascontiguousarray(y.astype(np.float32))


# revision 17
# speedup vs baseline: 1.1040x; 1.1040x over previous
"""Trainium2 Bass kernel: memory-slot cross-attention (nn_LocalConstructorMulti).

Reference computation (per batch b):
    Q  = memory_slots @ Wq.T                      [slots, BD]    (shared over b)
    K  = hs_b @ Wk.T                              [S, BD]
    V  = hs_b @ Wv.T                              [S, BD]
    s  = (Q_h . K_h) / sqrt(HD)  + mask           [heads, slots, S]
    p  = softmax(s, axis=S)
    o  = p @ V_h                                  [heads, slots, HD]
    y  = concat_h(o) @ Wo.T                       [slots, HID]

Sharding: 8 cores = 4 batches x 2 head-groups (4 heads / 256 bottleneck dims
each).  Each core sees the full hidden states of its batch and a 256-wide
slice of Wk/Wv/Wo, computes the full softmax locally over its heads, and
produces a partial y (contribution of its 4 heads).  The host sums the two
partials per batch -- o_proj is linear in the per-head outputs.

Key layout/engine decisions (v2, rebuilt from the TimelineSim bottleneck
analysis of v1: PE sequencer + HWDGE were saturated by 2434 small matmuls and
519 small-descriptor DMAs while the PE array itself was 45% idle):

  - hs is shipped twice, host-preshuffled into [8 blocks, 128 ki, 16 pair,
    2 two, 512 col] so each block is ONE DMA with 16-32 KiB contiguous
    per-partition lines: an fp8 copy (x8 scale) feeding the K-projection and
    a bf16 copy feeding the V-projection.  19 DMAs total vs 519.
  - K-projection runs in fp8 with MatmulPerfMode.DoubleRow: stationary
    wk8[128,2,128] x moving hs8[128,2,512] contracts TWO 128-deep k-subtiles
    per instruction (256 instructions for the whole KT).  fp8 noise on the
    K side is attenuated ~64x by the near-uniform softmax, contributing
    ~0.1% to the output.  Wk is pre-scaled x64 so its N(0, 1/4096) entries
    sit in e4m3's normal range; the combined 512x scale is folded into Q.
  - V-projection stays bf16 (V noise passes straight into the output):
    stationary hs blocks [128,128] x moving wv [128,256], PSUM-accumulated
    over all 32 k-subtiles, 1024 instructions at full 128x128x256 occupancy.
  - scores are built per 128-row tile with TWO head-pairs stacked on
    partitions: stationary kt[128, 128rows] x moving block-diagonal
    q2[128, 16] -> 64 matmuls; mask is a per-partition bias fused into Exp.
  - o = p^T @ V_aug runs as ONE 32-instruction PSUM chain: stationary
    pt[128, 4h*8n] x moving v[128, 4h*65] computes all 4 heads at once
    (the off-diagonal head blocks are discarded); the ones-column yields
    sum(p) for free, normalization is a per-partition scalar multiply.
  - o_proj is emitted transposed: yT[8, 4096] = ot[bd,8]^T @ woT[bd,4096] in
    16 mov-512 matmuls; the host adds the two head-group partials per batch.
  - Q (8x512, 0.02% of FLOPs) is computed on host and shipped pre-scaled as
    the block-diagonal q2 operand; every PSUM accumulator owns a full bank
    (PE-W + DVE/ACT-R same-bank erratum).
"""

import sys

if "/opt/trn_rl_repo" not in sys.path:
    sys.path.insert(0, "/opt/trn_rl_repo")

import ml_dtypes
import numpy as np

import concourse.bass as bass  # noqa: F401  (AP helpers)
import concourse.mybir as mybir
import concourse.tile as tile
from concourse import bacc
from concourse.bass_utils import run_bass_kernel_spmd
from concourse.masks import make_identity

BF16 = mybir.dt.bfloat16
FP8 = mybir.dt.float8e4
F32 = mybir.dt.float32
npbf16 = ml_dtypes.bfloat16
npfp8 = ml_dtypes.float8_e4m3

B, S, HID = 4, 4096, 4096
SLOTS, HEADS, BD = 8, 8, 512
HD = BD // HEADS  # 64
N_CORES = 8
GROUPS = N_CORES // B  # head-groups per batch
HPC = HEADS // GROUPS  # heads per core = 4
BDC = HPC * HD  # bottleneck slice per core = 256
MASK_NEG = -30000.0

HS_SCALE = 8.0  # fp8 hs pre-scale (lifts N(0,1) out of e4m3 subnormals)
WK_SCALE = 64.0  # fp8 Wk pre-scale (Wk entries ~ N(0, 1/4096))
# total score scale 1/sqrt(HD) divided back out of the device-side K product
Q_SCALE = 1.0 / (np.sqrt(HD) * HS_SCALE * WK_SCALE)

NBLK = 8  # column blocks of the sequence
CBLK = S // NBLK  # 512 columns per block
NPAIR = HID // 256  # 16 k-subtile pairs
NRT = S // 128  # 32 row tiles

# test.py can flip this to capture an NTFF profile; harness never touches it.
TRACE = False
TRACE_CORES = None
LAST_RESULT = None

_cache = {}


def _build_module():
    """Emit + compile the single-core Bass module (same NEFF on all cores)."""
    nc = bacc.Bacc("TRN2", target_bir_lowering=False, debug=False, num_devices=N_CORES)

    hs8T = nc.dram_tensor("hs8T", [NBLK, 128, NPAIR, 2, CBLK], FP8, kind="ExternalInput").ap()
    hsbT = nc.dram_tensor("hsbT", [NBLK, 128, NPAIR, 2, CBLK], BF16, kind="ExternalInput").ap()
    wk8T = nc.dram_tensor("wk8T", [128, NPAIR, 2, BDC], FP8, kind="ExternalInput").ap()
    wvT = nc.dram_tensor("wvT", [128, NPAIR, 2, BDC], BF16, kind="ExternalInput").ap()
    woT = nc.dram_tensor("woT", [128, 2, HID], BF16, kind="ExternalInput").ap()
    q2T = nc.dram_tensor("q2T", [128, 2, 2 * SLOTS], BF16, kind="ExternalInput").ap()
    mbT = nc.dram_tensor("mbT", [128, NRT], F32, kind="ExternalInput").ap()
    ypT = nc.dram_tensor("ypT", [SLOTS, HID], F32, kind="ExternalOutput").ap()

    DR = mybir.MatmulPerfMode.DoubleRow

    with tile.TileContext(nc) as tc:
        with (
            tc.tile_pool(name="consts", bufs=1) as consts,
            tc.tile_pool(name="hs8p", bufs=2) as hs8p,
            tc.tile_pool(name="hsbp", bufs=2) as hsbp,
        ):
            # ---- resident weights / tables -------------------------------
            wk8_sb = consts.tile([128, NPAIR, 2, BDC], FP8)
            nc.sync.dma_start(out=wk8_sb, in_=wk8T)
            wv_sb = consts.tile([128, NPAIR, 2, BDC], BF16)
            nc.sync.dma_start(out=wv_sb, in_=wvT)
            # wo/q2/mb are DMA'd after block 0's hs tiles (they're needed
            # only by the lagged attention / final o_proj)
            wo_sb = consts.tile([128, 2, HID], BF16)
            q2_sb = consts.tile([128, 2, 2 * SLOTS], BF16)
            mb_sb = consts.tile([128, NRT], F32)
            ident = consts.tile([128, 128], BF16)
            make_identity(nc, ident)

            # ---- persistent intermediates --------------------------------
            kt_sb = consts.tile([128, 2, S], BF16)  # K.T x512  [bd, rows]
            v_sb = consts.tile([128, NRT, HPC, HD + 1], BF16)  # V rows + ones
            nc.vector.memset(v_sb[:, :, :, HD : HD + 1], 1.0)
            # exp(scores).T, head stride padded to 32 so each head's o-block
            # lands on a 32-partition boundary (engine partition-offset rule)
            pt_sb = consts.tile([128, NRT, HPC, 32], BF16)
            nc.vector.memset(pt_sb, 0.0)

            # ---- K/V projections + lagged attention, one streamed pass ---
            # attention for block b-1 (scores -> exp -> o-chain partial) is
            # emitted after block b's projections, so the PE never waits on
            # the ACT exp round-trip and the old 15-20us serial tail folds
            # into the DMA/PE overlap window.
            RPB = CBLK // 128  # rowtiles per block

            def emit_attn(blk):
                for i in range(blk * RPB, (blk + 1) * RPB):
                    s_ps = sps.tile([128, 512], F32, tag="s")
                    for m2 in range(2):
                        nc.tensor.matmul(
                            s_ps[:, m2 * 16 : (m2 + 1) * 16],
                            kt_sb[:, m2, i * 128 : (i + 1) * 128],
                            q2_sb[:, m2, :],
                            start=True,
                            stop=True,
                        )
                    nc.scalar.activation(
                        out=pt_sb[:, i, :, 0:SLOTS],
                        in_=s_ps[:, 0 : HPC * SLOTS].rearrange(
                            "p (h n) -> p h n", h=HPC
                        ),
                        func=mybir.ActivationFunctionType.Exp,
                        bias=mb_sb[:, i : i + 1],
                        scale=1.0,
                    )
                for i in range(blk * RPB, (blk + 1) * RPB):
                    nc.tensor.matmul(
                        o_ps[:, 0 : HPC * (HD + 1)],
                        pt_sb[:, i, :, :],
                        v_sb[:, i, :, :],
                        start=(i == 0),
                        stop=(i == NRT - 1),
                    )

            oc = consts.tile([SLOTS, HPC, HD + 1], F32)
            with (
                tc.tile_pool(name="ktps", bufs=2, space="PSUM") as ktps,
                tc.tile_pool(name="vps", bufs=2, space="PSUM") as vps,
                tc.tile_pool(name="sps", bufs=2, space="PSUM") as sps,
                tc.tile_pool(name="ops", bufs=1, space="PSUM") as ops,
            ):
                o_ps = ops.tile([128, 512], F32)
                for blk in range(NBLK):
                    # hs DMAs arrive in pair-group chunks so the K/V chains
                    # start consuming before the whole block has landed
                    hs8_t = hs8p.tile([128, NPAIR, 2, CBLK], FP8, tag="hs8")
                    for q in range(4):
                        h = NPAIR // 4
                        nc.sync.dma_start(
                            out=hs8_t[:, q * h : (q + 1) * h],
                            in_=hs8T[blk][:, q * h : (q + 1) * h],
                        )
                    hsb_t = hsbp.tile([128, NPAIR, 2, CBLK], BF16, tag="hsb")
                    for q in range(8):
                        h = NPAIR // 8
                        nc.sync.dma_start(
                            out=hsb_t[:, q * h : (q + 1) * h],
                            in_=hsbT[blk][:, q * h : (q + 1) * h],
                        )
                    if blk == 0:
                        nc.sync.dma_start(out=q2_sb, in_=q2T)
                        nc.sync.dma_start(out=mb_sb, in_=mbT)
                    if blk == NBLK - 1:
                        # wo is first needed by o_proj at the very end; keep
                        # it out of the hs stream so it causes no PE bubble
                        nc.sync.dma_start(out=wo_sb, in_=woT)
                    # KT chains: fp8 DoubleRow, 2 k-subtiles per instruction
                    for m2 in range(2):
                        kt_ps = ktps.tile([128, CBLK], F32, tag="kt")
                        for p in range(NPAIR):
                            nc.tensor.matmul(
                                kt_ps,
                                wk8_sb[:, p, :, m2 * 128 : (m2 + 1) * 128],
                                hs8_t[:, p, :, :],
                                start=(p == 0),
                                stop=(p == NPAIR - 1),
                                perf_mode=DR,
                            )
                        nc.scalar.copy(
                            out=kt_sb[:, m2, blk * CBLK : (blk + 1) * CBLK],
                            in_=kt_ps,
                        )
                    # V chains: bf16, stationary hs blocks, full-bank psum
                    for r in range(RPB):
                        v_ps = vps.tile([128, 512], F32, tag="v")
                        for p in range(NPAIR):
                            for t in range(2):
                                nc.tensor.matmul(
                                    v_ps[:, 0:BDC],
                                    hsb_t[:, p, t, r * 128 : (r + 1) * 128],
                                    wv_sb[:, p, t, :],
                                    start=(p == 0 and t == 0),
                                    stop=(p == NPAIR - 1 and t == 1),
                                )
                        rt = blk * RPB + r
                        nc.vector.tensor_copy(
                            out=v_sb[:, rt, :, 0:HD],
                            in_=v_ps[:, 0:BDC].rearrange("p (h d) -> p h d", h=HPC),
                        )
                    if blk > 0:
                        emit_attn(blk - 1)
                emit_attn(NBLK - 1)
                for h in range(HPC):
                    eng = nc.scalar if h % 2 == 0 else nc.vector
                    copy = eng.copy if h % 2 == 0 else eng.tensor_copy
                    copy(
                        out=oc[:, h, :],
                        in_=o_ps[h * 32 : h * 32 + SLOTS,
                                 h * (HD + 1) : (h + 1) * (HD + 1)],
                    )

            # normalize: o / sum(p) via the ones column
            recip = consts.tile([SLOTS, HPC], F32)
            o_slot = consts.tile([SLOTS, BDC], BF16)
            for h in range(HPC):
                nc.vector.reciprocal(
                    out=recip[:, h : h + 1], in_=oc[:, h, HD : HD + 1]
                )
                nc.vector.tensor_scalar_mul(
                    out=o_slot[:, h * HD : (h + 1) * HD],
                    in0=oc[:, h, 0:HD],
                    scalar1=recip[:, h : h + 1],
                )

            # ---- transpose o to [bd, slots] ------------------------------
            ot_sb = consts.tile([128, 2, SLOTS], BF16)
            with tc.tile_pool(name="tps", bufs=2, space="PSUM") as tps:
                for j in range(2):
                    t_ps = tps.tile([128, 1024], BF16, tag="t")
                    nc.tensor.transpose(
                        t_ps[:, 0:SLOTS],
                        o_slot[:, j * 128 : (j + 1) * 128],
                        ident[:SLOTS, :SLOTS],
                    )
                    nc.scalar.copy(out=ot_sb[:, j, :], in_=t_ps[:, 0:SLOTS])

            # ---- partial o_proj, transposed: yT = ot^T @ woT -------------
            y_sb = consts.tile([SLOTS, HID], F32)
            with tc.tile_pool(name="yps", bufs=2, space="PSUM") as yps:
                for seg in range(HID // 512):
                    y_ps = yps.tile([SLOTS, 512], F32, tag="y")
                    for j in range(2):
                        nc.tensor.matmul(
                            y_ps,
                            ot_sb[:, j, :],
                            wo_sb[:, j, seg * 512 : (seg + 1) * 512],
                            start=(j == 0),
                            stop=(j == 1),
                        )
                    if seg % 2 == 0:
                        nc.scalar.copy(
                            out=y_sb[:, seg * 512 : (seg + 1) * 512], in_=y_ps
                        )
                    else:
                        nc.vector.tensor_copy(
                            out=y_sb[:, seg * 512 : (seg + 1) * 512], in_=y_ps
                        )
                nc.sync.dma_start(out=ypT, in_=y_sb)

    nc.compile()
    return nc


def _get_module():
    if "m" not in _cache:
        _cache["m"] = _build_module()
    return _cache["m"]


def _shuffle_hs(hsT_np, dtype, scale=1.0):
    """[HID, S] -> [NBLK, 128, NPAIR, 2, CBLK] with the (pair, two, ki)
    k-decomposition on axis 0 and (blk, col) on axis 1."""
    a = hsT_np.reshape(NPAIR, 2, 128, NBLK, CBLK)
    a = a.transpose(3, 2, 0, 1, 4)  # blk, ki, pair, two, col
    if scale != 1.0:
        a = a * np.float32(scale)
    return np.ascontiguousarray(a.astype(dtype))


def _prep_in_maps(hs, mask, ms, Wq, Wk, Wv, Wo):
    """Shard the full inputs into 8 per-core input maps (host-side)."""
    WkT = Wk.T.astype(np.float32)  # [HID, BD]
    WvT = Wv.T.astype(np.float32)
    WoT = Wo.T.astype(np.float32)  # [BD, HID]
    Q = (ms @ Wq.T).astype(np.float32)  # [SLOTS, BD]

    hs8 = []
    hsb = []
    mb = []
    for b in range(B):
        hsT = np.ascontiguousarray(hs[b].T)  # [HID, S]
        hs8.append(_shuffle_hs(hsT, npfp8, HS_SCALE))
        hsb.append(_shuffle_hs(hsT, npbf16))
        mb.append(
            np.ascontiguousarray(
                np.where(mask[b] == 0, np.float32(MASK_NEG), np.float32(0.0))
                .astype(np.float32)
                .reshape(NRT, 128)
                .T
            )
        )

    in_maps = []
    for c in range(N_CORES):
        b, g = c // GROUPS, c % GROUPS
        sl = slice(g * BDC, (g + 1) * BDC)
        wk8 = (
            (WkT[:, sl] * np.float32(WK_SCALE))
            .reshape(NPAIR, 2, 128, BDC)
            .transpose(2, 0, 1, 3)
        )
        wv = WvT[:, sl].reshape(NPAIR, 2, 128, BDC).transpose(2, 0, 1, 3)
        wo = WoT[sl].reshape(2, 128, HID).transpose(1, 0, 2)
        q2 = np.zeros((128, 2, 2 * SLOTS), np.float32)
        for m2 in range(2):
            h0 = g * HPC + 2 * m2
            q2[0:64, m2, 0:SLOTS] = Q[:, h0 * HD : (h0 + 1) * HD].T * Q_SCALE
            q2[64:128, m2, SLOTS : 2 * SLOTS] = (
                Q[:, (h0 + 1) * HD : (h0 + 2) * HD].T * Q_SCALE
            )
        in_maps.append(
            {
                "hs8T": hs8[b],
                "hsbT": hsb[b],
                "wk8T": np.ascontiguousarray(wk8.astype(npfp8)),
                "wvT": np.ascontiguousarray(wv.astype(npbf16)),
                "woT": np.ascontiguousarray(wo.astype(npbf16)),
                "q2T": np.ascontiguousarray(q2.astype(npbf16)),
                "mbT": mb[b],
            }
        )
    return in_maps


def kernel(hidden_states, attention_mask, memory_slots, Wq, Wk, Wv, Wo):
    global LAST_RESULT
    hs = np.asarray(hidden_states, dtype=np.float32)
    mask = np.asarray(attention_mask)
    ms = np.asarray(memory_slots, dtype=np.float32)
    Wq = np.asarray(Wq, dtype=np.float32)
    Wk = np.asarray(Wk, dtype=np.float32)
    Wv = np.asarray(Wv, dtype=np.float32)
    Wo = np.asarray(Wo, dtype=np.float32)

    nc = _get_module()
    in_maps = _prep_in_maps(hs, mask, ms, Wq, Wk, Wv, Wo)

    kwargs = {}
    if TRACE:
        kwargs = {"trace": True}
        if TRACE_CORES is not None:
            kwargs["trace_cores"] = TRACE_CORES
    res = run_bass_kernel_spmd(nc, in_maps, core_ids=list(range(N_CORES)), **kwargs)
    LAST_RESULT = res

    yp = [r["ypT"] for r in res.results]  # each [SLOTS, HID] f32
    y = np.stack(
        [yp[GROUPS * b] + yp[GROUPS * b + 1] for b in range(B)], axis=0
    )
    return np.ascontiguousarray(y.astype(np.float32))


# revision 25
# speedup vs baseline: 1.1281x; 1.0218x over previous
"""Trainium2 Bass kernel: memory-slot cross-attention (nn_LocalConstructorMulti).

Reference computation (per batch b):
    Q  = memory_slots @ Wq.T                      [slots, BD]    (shared over b)
    K  = hs_b @ Wk.T                              [S, BD]
    V  = hs_b @ Wv.T                              [S, BD]
    s  = (Q_h . K_h) / sqrt(HD)  + mask           [heads, slots, S]
    p  = softmax(s, axis=S)
    o  = p @ V_h                                  [heads, slots, HD]
    y  = concat_h(o) @ Wo.T                       [slots, HID]

Sharding: 8 cores = 4 batches x 2 head-groups (4 heads / 256 bottleneck dims
each).  Each core sees the full hidden states of its batch and a 256-wide
slice of Wk/Wv/Wo, computes the full softmax locally over its heads, and
produces a partial y (contribution of its 4 heads).  The host sums the two
partials per batch -- o_proj is linear in the per-head outputs.

Key layout/engine decisions (v2, rebuilt from the TimelineSim bottleneck
analysis of v1: PE sequencer + HWDGE were saturated by 2434 small matmuls and
519 small-descriptor DMAs while the PE array itself was 45% idle):

  - hs is shipped twice, host-preshuffled into [8 blocks, 128 ki, 16 pair,
    2 two, 512 col] so each block is ONE DMA with 16-32 KiB contiguous
    per-partition lines: an fp8 copy (x8 scale) feeding the K-projection and
    a bf16 copy feeding the V-projection.  19 DMAs total vs 519.
  - K-projection runs in fp8 with MatmulPerfMode.DoubleRow: stationary
    wk8[128,2,128] x moving hs8[128,2,512] contracts TWO 128-deep k-subtiles
    per instruction (256 instructions for the whole KT).  fp8 noise on the
    K side is attenuated ~64x by the near-uniform softmax, contributing
    ~0.1% to the output.  Wk is pre-scaled x64 so its N(0, 1/4096) entries
    sit in e4m3's normal range; the combined 512x scale is folded into Q.
  - V-projection stays bf16 (V noise passes straight into the output):
    stationary hs blocks [128,128] x moving wv [128,256], PSUM-accumulated
    over all 32 k-subtiles, 1024 instructions at full 128x128x256 occupancy.
  - scores are built per 128-row tile with TWO head-pairs stacked on
    partitions: stationary kt[128, 128rows] x moving block-diagonal
    q2[128, 16] -> 64 matmuls; mask is a per-partition bias fused into Exp.
  - o = p^T @ V_aug runs as ONE 32-instruction PSUM chain: stationary
    pt[128, 4h*8n] x moving v[128, 4h*65] computes all 4 heads at once
    (the off-diagonal head blocks are discarded); the ones-column yields
    sum(p) for free, normalization is a per-partition scalar multiply.
  - o_proj is emitted transposed: yT[8, 4096] = ot[bd,8]^T @ woT[bd,4096] in
    16 mov-512 matmuls; the host adds the two head-group partials per batch.
  - Q (8x512, 0.02% of FLOPs) is computed on host and shipped pre-scaled as
    the block-diagonal q2 operand; every PSUM accumulator owns a full bank
    (PE-W + DVE/ACT-R same-bank erratum).
"""

import sys

if "/opt/trn_rl_repo" not in sys.path:
    sys.path.insert(0, "/opt/trn_rl_repo")

import ml_dtypes
import numpy as np

import concourse.bass as bass  # noqa: F401  (AP helpers)
import concourse.mybir as mybir
import concourse.tile as tile
from concourse import bacc
from concourse.bass_utils import run_bass_kernel_spmd
from concourse.masks import make_identity

BF16 = mybir.dt.bfloat16
FP8 = mybir.dt.float8e4
F32 = mybir.dt.float32
npbf16 = ml_dtypes.bfloat16
npfp8 = ml_dtypes.float8_e4m3

B, S, HID = 4, 4096, 4096
SLOTS, HEADS, BD = 8, 8, 512
HD = BD // HEADS  # 64
N_CORES = 8
GROUPS = N_CORES // B  # head-groups per batch
HPC = HEADS // GROUPS  # heads per core = 4
BDC = HPC * HD  # bottleneck slice per core = 256
MASK_NEG = -30000.0

HS_SCALE = 8.0  # fp8 hs pre-scale (lifts N(0,1) out of e4m3 subnormals)
WK_SCALE = 64.0  # fp8 Wk pre-scale (Wk entries ~ N(0, 1/4096))
# total score scale 1/sqrt(HD) divided back out of the device-side K product
Q_SCALE = 1.0 / (np.sqrt(HD) * HS_SCALE * WK_SCALE)

NBLK = 8  # column blocks of the sequence
CBLK = S // NBLK  # 512 columns per block
NPAIR = HID // 256  # 16 k-subtile pairs
NRT = S // 128  # 32 row tiles

# test.py can flip this to capture an NTFF profile; harness never touches it.
TRACE = False
TRACE_CORES = None
LAST_RESULT = None

_cache = {}


def _build_module():
    """Emit + compile the single-core Bass module (same NEFF on all cores)."""
    nc = bacc.Bacc("TRN2", target_bir_lowering=False, debug=False, num_devices=N_CORES)

    hs8T = nc.dram_tensor("hs8T", [NBLK, 128, NPAIR, 2, CBLK], FP8, kind="ExternalInput").ap()
    hsbT = nc.dram_tensor("hsbT", [NBLK, 128, NPAIR, 2, CBLK], BF16, kind="ExternalInput").ap()
    wk8T = nc.dram_tensor("wk8T", [128, NPAIR, 2, BDC], FP8, kind="ExternalInput").ap()
    wvT = nc.dram_tensor("wvT", [128, NPAIR, 2, BDC], BF16, kind="ExternalInput").ap()
    woT = nc.dram_tensor("woT", [128, 2, HID], BF16, kind="ExternalInput").ap()
    q2T = nc.dram_tensor("q2T", [128, 2, 2 * SLOTS], BF16, kind="ExternalInput").ap()
    mbT = nc.dram_tensor("mbT", [128, NRT], F32, kind="ExternalInput").ap()
    ypT = nc.dram_tensor("ypT", [SLOTS, HID], F32, kind="ExternalOutput").ap()

    DR = mybir.MatmulPerfMode.DoubleRow

    with tile.TileContext(nc) as tc:
        with (
            tc.tile_pool(name="consts", bufs=1) as consts,
            tc.tile_pool(name="hs8p", bufs=2) as hs8p,
            tc.tile_pool(name="hsbp", bufs=2) as hsbp,
        ):
            # ---- resident weights / tables -------------------------------
            wk8_sb = consts.tile([128, NPAIR, 2, BDC], FP8)
            wv_sb = consts.tile([128, NPAIR, 2, BDC], BF16)
            # wo/q2/mb are DMA'd after block 0's hs tiles (they're needed
            # only by the lagged attention / final o_proj)
            wo_sb = consts.tile([128, 2, HID], BF16)
            q2_sb = consts.tile([128, 2, 2 * SLOTS], BF16)
            mb_sb = consts.tile([128, NRT], F32)
            ident = consts.tile([128, 128], BF16)
            make_identity(nc, ident)

            # ---- persistent intermediates --------------------------------
            kt_sb = consts.tile([128, 2, S], BF16)  # K.T x512  [bd, rows]
            v_sb = consts.tile([128, NRT, HPC, HD + 1], BF16)  # V rows + ones
            nc.vector.memset(v_sb[:, :, :, HD : HD + 1], 1.0)
            # exp(scores).T, head stride padded to 32 so each head's o-block
            # lands on a 32-partition boundary (engine partition-offset rule)
            pt_sb = consts.tile([128, NRT, HPC, 32], BF16)
            nc.vector.memset(pt_sb, 0.0)

            # ---- K/V projections + lagged attention, one streamed pass ---
            # attention for block b-1 (scores -> exp -> o-chain partial) is
            # emitted after block b's projections, so the PE never waits on
            # the ACT exp round-trip and the old 15-20us serial tail folds
            # into the DMA/PE overlap window.
            RPB = CBLK // 128  # rowtiles per block

            def emit_attn(blk):
                for i in range(blk * RPB, (blk + 1) * RPB):
                    s_ps = sps.tile([128, 512], F32, tag="s")
                    for m2 in range(2):
                        nc.tensor.matmul(
                            s_ps[:, m2 * 16 : (m2 + 1) * 16],
                            kt_sb[:, m2, i * 128 : (i + 1) * 128],
                            q2_sb[:, m2, :],
                            start=True,
                            stop=True,
                        )
                    nc.scalar.activation(
                        out=pt_sb[:, i, :, 0:SLOTS],
                        in_=s_ps[:, 0 : HPC * SLOTS].rearrange(
                            "p (h n) -> p h n", h=HPC
                        ),
                        func=mybir.ActivationFunctionType.Exp,
                        bias=mb_sb[:, i : i + 1],
                        scale=1.0,
                    )
                for i in range(blk * RPB, (blk + 1) * RPB):
                    nc.tensor.matmul(
                        o_ps[:, 0 : HPC * (HD + 1)],
                        pt_sb[:, i, :, :],
                        v_sb[:, i, :, :],
                        start=(i == 0),
                        stop=(i == NRT - 1),
                    )

            oc = consts.tile([SLOTS, HPC, HD + 1], F32)
            with (
                tc.tile_pool(name="ktps", bufs=2, space="PSUM") as ktps,
                tc.tile_pool(name="vps", bufs=2, space="PSUM") as vps,
                tc.tile_pool(name="sps", bufs=2, space="PSUM") as sps,
                tc.tile_pool(name="ops", bufs=1, space="PSUM") as ops,
            ):
                o_ps = ops.tile([128, 512], F32)
                for blk in range(NBLK):
                    # hs DMAs arrive in pair-group chunks so the K/V chains
                    # start consuming before the whole block has landed;
                    # the V-path operands stream first (V is the long pole)
                    hsb_t = hsbp.tile([128, NPAIR, 2, CBLK], BF16, tag="hsb")
                    for q in range(8):
                        h = NPAIR // 8
                        if blk == 0 and q < 8:
                            # wv pair-chunks ride along with the first hsb
                            # chunks: the V chain consumes pairs in order
                            nc.sync.dma_start(
                                out=wv_sb[:, q * h : (q + 1) * h],
                                in_=wvT[:, q * h : (q + 1) * h],
                            )
                        nc.sync.dma_start(
                            out=hsb_t[:, q * h : (q + 1) * h],
                            in_=hsbT[blk][:, q * h : (q + 1) * h],
                        )
                    if blk == 0:
                        nc.sync.dma_start(out=wk8_sb, in_=wk8T)
                    hs8_t = hs8p.tile([128, NPAIR, 2, CBLK], FP8, tag="hs8")
                    for q in range(4):
                        h = NPAIR // 4
                        nc.sync.dma_start(
                            out=hs8_t[:, q * h : (q + 1) * h],
                            in_=hs8T[blk][:, q * h : (q + 1) * h],
                        )
                    if blk == 0:
                        nc.sync.dma_start(out=q2_sb, in_=q2T)
                        nc.sync.dma_start(out=mb_sb, in_=mbT)
                    if blk == NBLK - 1:
                        # wo is first needed by o_proj at the very end; keep
                        # it out of the hs stream so it causes no PE bubble
                        nc.sync.dma_start(out=wo_sb, in_=woT)
                    # V chains: bf16, stationary hs blocks, full-bank psum
                    for r in range(RPB):
                        v_ps = vps.tile([128, 512], F32, tag="v")
                        for p in range(NPAIR):
                            for t in range(2):
                                nc.tensor.matmul(
                                    v_ps[:, 0:BDC],
                                    hsb_t[:, p, t, r * 128 : (r + 1) * 128],
                                    wv_sb[:, p, t, :],
                                    start=(p == 0 and t == 0),
                                    stop=(p == NPAIR - 1 and t == 1),
                                )
                        rt = blk * RPB + r
                        nc.vector.tensor_copy(
                            out=v_sb[:, rt, :, 0:HD],
                            in_=v_ps[:, 0:BDC].rearrange("p (h d) -> p h d", h=HPC),
                        )
                    # KT chains: fp8 DoubleRow, 2 k-subtiles per instruction
                    for m2 in range(2):
                        kt_ps = ktps.tile([128, CBLK], F32, tag="kt")
                        for p in range(NPAIR):
                            nc.tensor.matmul(
                                kt_ps,
                                wk8_sb[:, p, :, m2 * 128 : (m2 + 1) * 128],
                                hs8_t[:, p, :, :],
                                start=(p == 0),
                                stop=(p == NPAIR - 1),
                                perf_mode=DR,
                            )
                        nc.scalar.copy(
                            out=kt_sb[:, m2, blk * CBLK : (blk + 1) * CBLK],
                            in_=kt_ps,
                        )
                    emit_attn(blk)
                for h in range(HPC):
                    eng = nc.scalar if h % 2 == 0 else nc.vector
                    copy = eng.copy if h % 2 == 0 else eng.tensor_copy
                    copy(
                        out=oc[:, h, :],
                        in_=o_ps[h * 32 : h * 32 + SLOTS,
                                 h * (HD + 1) : (h + 1) * (HD + 1)],
                    )

            # normalize: o / sum(p) via the ones column
            recip = consts.tile([SLOTS, HPC], F32)
            o_slot = consts.tile([SLOTS, BDC], BF16)
            nc.vector.reciprocal(out=recip, in_=oc[:, :, HD])
            for h in range(HPC):
                nc.vector.tensor_scalar_mul(
                    out=o_slot[:, h * HD : (h + 1) * HD],
                    in0=oc[:, h, 0:HD],
                    scalar1=recip[:, h : h + 1],
                )

            # ---- transpose o to [bd, slots] ------------------------------
            ot_sb = consts.tile([128, 2, SLOTS], BF16)
            with tc.tile_pool(name="tps", bufs=2, space="PSUM") as tps:
                for j in range(2):
                    t_ps = tps.tile([128, 1024], BF16, tag="t")
                    nc.tensor.transpose(
                        t_ps[:, 0:SLOTS],
                        o_slot[:, j * 128 : (j + 1) * 128],
                        ident[:SLOTS, :SLOTS],
                    )
                    nc.scalar.copy(out=ot_sb[:, j, :], in_=t_ps[:, 0:SLOTS])

            # ---- partial o_proj, transposed: yT = ot^T @ woT -------------
            # drains alternate ACT/DVE so neither engine paces the phase;
            # each 512-seg is DMA'd out as soon as its copy lands
            y_sb = consts.tile([SLOTS, HID], F32)
            with tc.tile_pool(name="yps", bufs=4, space="PSUM") as yps:
                for seg in range(HID // 512):
                    y_ps = yps.tile([SLOTS, 512], F32, tag="y")
                    for j in range(2):
                        nc.tensor.matmul(
                            y_ps,
                            ot_sb[:, j, :],
                            wo_sb[:, j, seg * 512 : (seg + 1) * 512],
                            start=(j == 0),
                            stop=(j == 1),
                        )
                    if seg % 2 == 0:
                        nc.scalar.copy(
                            out=y_sb[:, seg * 512 : (seg + 1) * 512], in_=y_ps
                        )
                    else:
                        nc.vector.tensor_copy(
                            out=y_sb[:, seg * 512 : (seg + 1) * 512], in_=y_ps
                        )
                    if seg % 4 == 3:
                        nc.sync.dma_start(
                            out=ypT[:, (seg - 3) * 512 : (seg + 1) * 512],
                            in_=y_sb[:, (seg - 3) * 512 : (seg + 1) * 512],
                        )

    nc.compile()
    return nc


def _get_module():
    if "m" not in _cache:
        _cache["m"] = _build_module()
    return _cache["m"]


def _shuffle_hs(hsT_np, dtype, scale=1.0):
    """[HID, S] -> [NBLK, 128, NPAIR, 2, CBLK] with the (pair, two, ki)
    k-decomposition on axis 0 and (blk, col) on axis 1."""
    a = hsT_np.reshape(NPAIR, 2, 128, NBLK, CBLK)
    a = a.transpose(3, 2, 0, 1, 4)  # blk, ki, pair, two, col
    if scale != 1.0:
        a = a * np.float32(scale)
    return np.ascontiguousarray(a.astype(dtype))


def _prep_in_maps(hs, mask, ms, Wq, Wk, Wv, Wo):
    """Shard the full inputs into 8 per-core input maps (host-side)."""
    WkT = Wk.T.astype(np.float32)  # [HID, BD]
    WvT = Wv.T.astype(np.float32)
    WoT = Wo.T.astype(np.float32)  # [BD, HID]
    Q = (ms @ Wq.T).astype(np.float32)  # [SLOTS, BD]

    hs8 = []
    hsb = []
    mb = []
    for b in range(B):
        hsT = np.ascontiguousarray(hs[b].T)  # [HID, S]
        hs8.append(_shuffle_hs(hsT, npfp8, HS_SCALE))
        hsb.append(_shuffle_hs(hsT, npbf16))
        mb.append(
            np.ascontiguousarray(
                np.where(mask[b] == 0, np.float32(MASK_NEG), np.float32(0.0))
                .astype(np.float32)
                .reshape(NRT, 128)
                .T
            )
        )

    in_maps = []
    for c in range(N_CORES):
        b, g = c // GROUPS, c % GROUPS
        sl = slice(g * BDC, (g + 1) * BDC)
        wk8 = (
            (WkT[:, sl] * np.float32(WK_SCALE))
            .reshape(NPAIR, 2, 128, BDC)
            .transpose(2, 0, 1, 3)
        )
        wv = WvT[:, sl].reshape(NPAIR, 2, 128, BDC).transpose(2, 0, 1, 3)
        wo = WoT[sl].reshape(2, 128, HID).transpose(1, 0, 2)
        q2 = np.zeros((128, 2, 2 * SLOTS), np.float32)
        for m2 in range(2):
            h0 = g * HPC + 2 * m2
            q2[0:64, m2, 0:SLOTS] = Q[:, h0 * HD : (h0 + 1) * HD].T * Q_SCALE
            q2[64:128, m2, SLOTS : 2 * SLOTS] = (
                Q[:, (h0 + 1) * HD : (h0 + 2) * HD].T * Q_SCALE
            )
        in_maps.append(
            {
                "hs8T": hs8[b],
                "hsbT": hsb[b],
                "wk8T": np.ascontiguousarray(wk8.astype(npfp8)),
                "wvT": np.ascontiguousarray(wv.astype(npbf16)),
                "woT": np.ascontiguousarray(wo.astype(npbf16)),
                "q2T": np.ascontiguousarray(q2.astype(npbf16)),
                "mbT": mb[b],
            }
        )
    return in_maps


def kernel(hidden_states, attention_mask, memory_slots, Wq, Wk, Wv, Wo):
    global LAST_RESULT
    hs = np.asarray(hidden_states, dtype=np.float32)
    mask = np.asarray(attention_mask)
    ms = np.asarray(memory_slots, dtype=np.float32)
    Wq = np.asarray(Wq, dtype=np.float32)
    Wk = np.asarray(Wk, dtype=np.float32)
    Wv = np.asarray(Wv, dtype=np.float32)
    Wo = np.asarray(Wo, dtype=np.float32)

    nc = _get_module()
    in_maps = _prep_in_maps(hs, mask, ms, Wq, Wk, Wv, Wo)

    kwargs = {}
    if TRACE:
        kwargs = {"trace": True}
        if TRACE_CORES is not None:
            kwargs["trace_cores"] = TRACE_CORES
    res = run_bass_kernel_spmd(nc, in_maps, core_ids=list(range(N_CORES)), **kwargs)
    LAST_RESULT = res

    yp = [r["ypT"] for r in res.results]  # each [SLOTS, HID] f32
    y = np.stack(
        [yp[GROUPS * b] + yp[GROUPS * b + 1] for b in range(B)], axis=0
    )
    return np.ascontiguousarray(y.astype(np.float32))


# revision 28
# speedup vs baseline: 1.1571x; 1.0257x over previous
"""Trainium2 Bass kernel: memory-slot cross-attention (nn_LocalConstructorMulti).

Reference computation (per batch b):
    Q  = memory_slots @ Wq.T                      [slots, BD]    (shared over b)
    K  = hs_b @ Wk.T                              [S, BD]
    V  = hs_b @ Wv.T                              [S, BD]
    s  = (Q_h . K_h) / sqrt(HD)  + mask           [heads, slots, S]
    p  = softmax(s, axis=S)
    o  = p @ V_h                                  [heads, slots, HD]
    y  = concat_h(o) @ Wo.T                       [slots, HID]

Sharding: 8 cores = 4 batches x 2 head-groups (4 heads / 256 bottleneck dims
each).  Each core sees the full hidden states of its batch and a 256-wide
slice of Wk/Wv/Wo, computes the full softmax locally over its heads, and
produces a partial y (contribution of its 4 heads).  The host sums the two
partials per batch -- o_proj is linear in the per-head outputs.

Key layout/engine decisions (v2, rebuilt from the TimelineSim bottleneck
analysis of v1: PE sequencer + HWDGE were saturated by 2434 small matmuls and
519 small-descriptor DMAs while the PE array itself was 45% idle):

  - hs is shipped twice, host-preshuffled into [8 blocks, 128 ki, 16 pair,
    2 two, 512 col] so each block is ONE DMA with 16-32 KiB contiguous
    per-partition lines: an fp8 copy (x8 scale) feeding the K-projection and
    a bf16 copy feeding the V-projection.  19 DMAs total vs 519.
  - K-projection runs in fp8 with MatmulPerfMode.DoubleRow: stationary
    wk8[128,2,128] x moving hs8[128,2,512] contracts TWO 128-deep k-subtiles
    per instruction (256 instructions for the whole KT).  fp8 noise on the
    K side is attenuated ~64x by the near-uniform softmax, contributing
    ~0.1% to the output.  Wk is pre-scaled x64 so its N(0, 1/4096) entries
    sit in e4m3's normal range; the combined 512x scale is folded into Q.
  - V-projection stays bf16 (V noise passes straight into the output):
    stationary hs blocks [128,128] x moving wv [128,256], PSUM-accumulated
    over all 32 k-subtiles, 1024 instructions at full 128x128x256 occupancy.
  - scores are built per 128-row tile with TWO head-pairs stacked on
    partitions: stationary kt[128, 128rows] x moving block-diagonal
    q2[128, 16] -> 64 matmuls; mask is a per-partition bias fused into Exp.
  - o = p^T @ V_aug runs as ONE 32-instruction PSUM chain: stationary
    pt[128, 4h*8n] x moving v[128, 4h*65] computes all 4 heads at once
    (the off-diagonal head blocks are discarded); the ones-column yields
    sum(p) for free, normalization is a per-partition scalar multiply.
  - o_proj is emitted transposed: yT[8, 4096] = ot[bd,8]^T @ woT[bd,4096] in
    16 mov-512 matmuls; the host adds the two head-group partials per batch.
  - Q (8x512, 0.02% of FLOPs) is computed on host and shipped pre-scaled as
    the block-diagonal q2 operand; every PSUM accumulator owns a full bank
    (PE-W + DVE/ACT-R same-bank erratum).
"""

import sys

if "/opt/trn_rl_repo" not in sys.path:
    sys.path.insert(0, "/opt/trn_rl_repo")

import ml_dtypes
import numpy as np

import concourse.bass as bass  # noqa: F401  (AP helpers)
import concourse.mybir as mybir
import concourse.tile as tile
from concourse import bacc
from concourse.bass_utils import run_bass_kernel_spmd
from concourse.masks import make_identity

BF16 = mybir.dt.bfloat16
FP8 = mybir.dt.float8e4
F32 = mybir.dt.float32
npbf16 = ml_dtypes.bfloat16
npfp8 = ml_dtypes.float8_e4m3

B, S, HID = 4, 4096, 4096
SLOTS, HEADS, BD = 8, 8, 512
HD = BD // HEADS  # 64
N_CORES = 8
GROUPS = N_CORES // B  # head-groups per batch
HPC = HEADS // GROUPS  # heads per core = 4
BDC = HPC * HD  # bottleneck slice per core = 256
MASK_NEG = -30000.0

# hs8 is cast on-device from the bf16 copy (unscaled: K-path noise is
# attenuated ~64x by the softmax, so e4m3 subnormal loss is irrelevant);
# all fp8 range scaling lives in Wk (N(0, 1/4096) entries x512 -> N(0, 8))
WK_SCALE = 512.0
# total score scale 1/sqrt(HD) divided back out of the device-side K product
Q_SCALE = 1.0 / (np.sqrt(HD) * WK_SCALE)

NBLK = 8  # column blocks of the sequence
CBLK = S // NBLK  # 512 columns per block
NPAIR = HID // 256  # 16 k-subtile pairs
NRT = S // 128  # 32 row tiles

# test.py can flip this to capture an NTFF profile; harness never touches it.
TRACE = False
TRACE_CORES = None
LAST_RESULT = None

_cache = {}


def _build_module():
    """Emit + compile the single-core Bass module (same NEFF on all cores)."""
    nc = bacc.Bacc("TRN2", target_bir_lowering=False, debug=False, num_devices=N_CORES)

    hsbT = nc.dram_tensor("hsbT", [NBLK, 128, NPAIR, 2, CBLK], BF16, kind="ExternalInput").ap()
    wk8T = nc.dram_tensor("wk8T", [128, NPAIR, 2, BDC], FP8, kind="ExternalInput").ap()
    wvT = nc.dram_tensor("wvT", [128, NPAIR, 2, BDC], BF16, kind="ExternalInput").ap()
    woT = nc.dram_tensor("woT", [128, 2, HID], BF16, kind="ExternalInput").ap()
    q2T = nc.dram_tensor("q2T", [128, 2, 2 * SLOTS], BF16, kind="ExternalInput").ap()
    mbT = nc.dram_tensor("mbT", [128, NRT], F32, kind="ExternalInput").ap()
    ypT = nc.dram_tensor("ypT", [SLOTS, HID], F32, kind="ExternalOutput").ap()

    DR = mybir.MatmulPerfMode.DoubleRow

    with tile.TileContext(nc) as tc:
        with (
            tc.tile_pool(name="consts", bufs=1) as consts,
            tc.tile_pool(name="hs8p", bufs=2) as hs8p,
            tc.tile_pool(name="hsbp", bufs=2) as hsbp,
        ):
            # ---- resident weights / tables -------------------------------
            wk8_sb = consts.tile([128, NPAIR, 2, BDC], FP8)
            wv_sb = consts.tile([128, NPAIR, 2, BDC], BF16)
            # wo/q2/mb are DMA'd after block 0's hs tiles (they're needed
            # only by the lagged attention / final o_proj)
            wo_sb = consts.tile([128, 2, HID], BF16)
            q2_sb = consts.tile([128, 2, 2 * SLOTS], BF16)
            mb_sb = consts.tile([128, NRT], F32)
            ident = consts.tile([128, 128], BF16)
            make_identity(nc, ident)

            # ---- persistent intermediates --------------------------------
            kt_sb = consts.tile([128, 2, S], BF16)  # K.T x512  [bd, rows]
            v_sb = consts.tile([128, NRT, HPC, HD + 1], BF16)  # V rows + ones
            nc.vector.memset(v_sb[:, :, :, HD : HD + 1], 1.0)
            # exp(scores).T, head stride padded to 32 so each head's o-block
            # lands on a 32-partition boundary (engine partition-offset rule)
            pt_sb = consts.tile([128, NRT, HPC, 32], BF16)
            nc.vector.memset(pt_sb, 0.0)

            # ---- K/V projections + lagged attention, one streamed pass ---
            # attention for block b-1 (scores -> exp -> o-chain partial) is
            # emitted after block b's projections, so the PE never waits on
            # the ACT exp round-trip and the old 15-20us serial tail folds
            # into the DMA/PE overlap window.
            RPB = CBLK // 128  # rowtiles per block

            def emit_attn(blk):
                for i in range(blk * RPB, (blk + 1) * RPB):
                    s_ps = sps.tile([128, 512], F32, tag="s")
                    for m2 in range(2):
                        nc.tensor.matmul(
                            s_ps[:, m2 * 16 : (m2 + 1) * 16],
                            kt_sb[:, m2, i * 128 : (i + 1) * 128],
                            q2_sb[:, m2, :],
                            start=True,
                            stop=True,
                        )
                    nc.scalar.activation(
                        out=pt_sb[:, i, :, 0:SLOTS],
                        in_=s_ps[:, 0 : HPC * SLOTS].rearrange(
                            "p (h n) -> p h n", h=HPC
                        ),
                        func=mybir.ActivationFunctionType.Exp,
                        bias=mb_sb[:, i : i + 1],
                        scale=1.0,
                    )
                for i in range(blk * RPB, (blk + 1) * RPB):
                    nc.tensor.matmul(
                        o_ps[:, 0 : HPC * (HD + 1)],
                        pt_sb[:, i, :, :],
                        v_sb[:, i, :, :],
                        start=(i == 0),
                        stop=(i == NRT - 1),
                    )

            oc = consts.tile([SLOTS, HPC, HD + 1], F32)
            with (
                tc.tile_pool(name="ktps", bufs=2, space="PSUM") as ktps,
                tc.tile_pool(name="vps", bufs=2, space="PSUM") as vps,
                tc.tile_pool(name="sps", bufs=2, space="PSUM") as sps,
                tc.tile_pool(name="ops", bufs=1, space="PSUM") as ops,
            ):
                o_ps = ops.tile([128, 512], F32)
                for blk in range(NBLK):
                    # hs DMAs arrive in pair-group chunks so the K/V chains
                    # start consuming before the whole block has landed;
                    # the V-path operands stream first (V is the long pole)
                    hsb_t = hsbp.tile([128, NPAIR, 2, CBLK], BF16, tag="hsb")
                    hs8_t = hs8p.tile([128, NPAIR, 2, CBLK], FP8, tag="hs8")
                    cast_engines = (
                        nc.vector.tensor_copy,
                        nc.scalar.copy,
                    )
                    for q in range(8):
                        h = NPAIR // 8
                        if blk == 0:
                            # wv pair-chunks ride along with the first hsb
                            # chunks: the V chain consumes pairs in order
                            nc.sync.dma_start(
                                out=wv_sb[:, q * h : (q + 1) * h],
                                in_=wvT[:, q * h : (q + 1) * h],
                            )
                        nc.sync.dma_start(
                            out=hsb_t[:, q * h : (q + 1) * h],
                            in_=hsbT[blk][:, q * h : (q + 1) * h],
                        )
                        # hs8 is derived on-device: a casting copy on the
                        # otherwise-idle Pool/DVE/ACT engines replaces 16 MB
                        # of fp8 DMA traffic
                        cast_engines[q % 2](
                            out=hs8_t[:, q * h : (q + 1) * h],
                            in_=hsb_t[:, q * h : (q + 1) * h],
                        )
                    if blk == 0:
                        nc.sync.dma_start(out=wk8_sb, in_=wk8T)
                    if blk == 0:
                        nc.sync.dma_start(out=q2_sb, in_=q2T)
                        nc.sync.dma_start(out=mb_sb, in_=mbT)
                    if blk == NBLK - 1:
                        # wo is first needed by o_proj at the very end; keep
                        # it out of the hs stream so it causes no PE bubble
                        nc.sync.dma_start(out=wo_sb, in_=woT)
                    # V chains: bf16, stationary hs blocks, full-bank psum
                    for r in range(RPB):
                        v_ps = vps.tile([128, 512], F32, tag="v")
                        for p in range(NPAIR):
                            for t in range(2):
                                nc.tensor.matmul(
                                    v_ps[:, 0:BDC],
                                    hsb_t[:, p, t, r * 128 : (r + 1) * 128],
                                    wv_sb[:, p, t, :],
                                    start=(p == 0 and t == 0),
                                    stop=(p == NPAIR - 1 and t == 1),
                                )
                        rt = blk * RPB + r
                        nc.vector.tensor_copy(
                            out=v_sb[:, rt, :, 0:HD],
                            in_=v_ps[:, 0:BDC].rearrange("p (h d) -> p h d", h=HPC),
                        )
                    # KT chains: fp8 DoubleRow, 2 k-subtiles per instruction
                    for m2 in range(2):
                        kt_ps = ktps.tile([128, CBLK], F32, tag="kt")
                        for p in range(NPAIR):
                            nc.tensor.matmul(
                                kt_ps,
                                wk8_sb[:, p, :, m2 * 128 : (m2 + 1) * 128],
                                hs8_t[:, p, :, :],
                                start=(p == 0),
                                stop=(p == NPAIR - 1),
                                perf_mode=DR,
                            )
                        nc.scalar.copy(
                            out=kt_sb[:, m2, blk * CBLK : (blk + 1) * CBLK],
                            in_=kt_ps,
                        )
                    emit_attn(blk)
                for h in range(HPC):
                    eng = nc.scalar if h % 2 == 0 else nc.vector
                    copy = eng.copy if h % 2 == 0 else eng.tensor_copy
                    copy(
                        out=oc[:, h, :],
                        in_=o_ps[h * 32 : h * 32 + SLOTS,
                                 h * (HD + 1) : (h + 1) * (HD + 1)],
                    )

            # normalize: o / sum(p) via the ones column
            recip = consts.tile([SLOTS, HPC], F32)
            o_slot = consts.tile([SLOTS, BDC], BF16)
            nc.vector.reciprocal(out=recip, in_=oc[:, :, HD])
            for h in range(HPC):
                nc.vector.tensor_scalar_mul(
                    out=o_slot[:, h * HD : (h + 1) * HD],
                    in0=oc[:, h, 0:HD],
                    scalar1=recip[:, h : h + 1],
                )

            # ---- transpose o to [bd, slots] ------------------------------
            ot_sb = consts.tile([128, 2, SLOTS], BF16)
            with tc.tile_pool(name="tps", bufs=2, space="PSUM") as tps:
                for j in range(2):
                    t_ps = tps.tile([128, 1024], BF16, tag="t")
                    nc.tensor.transpose(
                        t_ps[:, 0:SLOTS],
                        o_slot[:, j * 128 : (j + 1) * 128],
                        ident[:SLOTS, :SLOTS],
                    )
                    nc.scalar.copy(out=ot_sb[:, j, :], in_=t_ps[:, 0:SLOTS])

            # ---- partial o_proj, transposed: yT = ot^T @ woT -------------
            # drains alternate ACT/DVE so neither engine paces the phase;
            # each 512-seg is DMA'd out as soon as its copy lands
            y_sb = consts.tile([SLOTS, HID], F32)
            with tc.tile_pool(name="yps", bufs=4, space="PSUM") as yps:
                for seg in range(HID // 512):
                    y_ps = yps.tile([SLOTS, 512], F32, tag="y")
                    for j in range(2):
                        nc.tensor.matmul(
                            y_ps,
                            ot_sb[:, j, :],
                            wo_sb[:, j, seg * 512 : (seg + 1) * 512],
                            start=(j == 0),
                            stop=(j == 1),
                        )
                    if seg % 2 == 0:
                        nc.scalar.copy(
                            out=y_sb[:, seg * 512 : (seg + 1) * 512], in_=y_ps
                        )
                    else:
                        nc.vector.tensor_copy(
                            out=y_sb[:, seg * 512 : (seg + 1) * 512], in_=y_ps
                        )
                    if seg % 4 == 3:
                        nc.sync.dma_start(
                            out=ypT[:, (seg - 3) * 512 : (seg + 1) * 512],
                            in_=y_sb[:, (seg - 3) * 512 : (seg + 1) * 512],
                        )

    nc.compile()
    return nc


def _get_module():
    if "m" not in _cache:
        _cache["m"] = _build_module()
    return _cache["m"]


def _shuffle_hs(hsT_np, dtype, scale=1.0):
    """[HID, S] -> [NBLK, 128, NPAIR, 2, CBLK] with the (pair, two, ki)
    k-decomposition on axis 0 and (blk, col) on axis 1."""
    a = hsT_np.reshape(NPAIR, 2, 128, NBLK, CBLK)
    a = a.transpose(3, 2, 0, 1, 4)  # blk, ki, pair, two, col
    if scale != 1.0:
        a = a * np.float32(scale)
    return np.ascontiguousarray(a.astype(dtype))


def _prep_in_maps(hs, mask, ms, Wq, Wk, Wv, Wo):
    """Shard the full inputs into 8 per-core input maps (host-side)."""
    WkT = Wk.T.astype(np.float32)  # [HID, BD]
    WvT = Wv.T.astype(np.float32)
    WoT = Wo.T.astype(np.float32)  # [BD, HID]
    Q = (ms @ Wq.T).astype(np.float32)  # [SLOTS, BD]

    hsb = []
    mb = []
    for b in range(B):
        hsT = np.ascontiguousarray(hs[b].T)  # [HID, S]
        hsb.append(_shuffle_hs(hsT, npbf16))
        mb.append(
            np.ascontiguousarray(
                np.where(mask[b] == 0, np.float32(MASK_NEG), np.float32(0.0))
                .astype(np.float32)
                .reshape(NRT, 128)
                .T
            )
        )

    in_maps = []
    for c in range(N_CORES):
        b, g = c // GROUPS, c % GROUPS
        sl = slice(g * BDC, (g + 1) * BDC)
        wk8 = (
            (WkT[:, sl] * np.float32(WK_SCALE))
            .reshape(NPAIR, 2, 128, BDC)
            .transpose(2, 0, 1, 3)
        )
        wv = WvT[:, sl].reshape(NPAIR, 2, 128, BDC).transpose(2, 0, 1, 3)
        wo = WoT[sl].reshape(2, 128, HID).transpose(1, 0, 2)
        q2 = np.zeros((128, 2, 2 * SLOTS), np.float32)
        for m2 in range(2):
            h0 = g * HPC + 2 * m2
            q2[0:64, m2, 0:SLOTS] = Q[:, h0 * HD : (h0 + 1) * HD].T * Q_SCALE
            q2[64:128, m2, SLOTS : 2 * SLOTS] = (
                Q[:, (h0 + 1) * HD : (h0 + 2) * HD].T * Q_SCALE
            )
        in_maps.append(
            {
                "hsbT": hsb[b],
                "wk8T": np.ascontiguousarray(wk8.astype(npfp8)),
                "wvT": np.ascontiguousarray(wv.astype(npbf16)),
                "woT": np.ascontiguousarray(wo.astype(npbf16)),
                "q2T": np.ascontiguousarray(q2.astype(npbf16)),
                "mbT": mb[b],
            }
        )
    return in_maps


def kernel(hidden_states, attention_mask, memory_slots, Wq, Wk, Wv, Wo):
    global LAST_RESULT
    hs = np.asarray(hidden_states, dtype=np.float32)
    mask = np.asarray(attention_mask)
    ms = np.asarray(memory_slots, dtype=np.float32)
    Wq = np.asarray(Wq, dtype=np.float32)
    Wk = np.asarray(Wk, dtype=np.float32)
    Wv = np.asarray(Wv, dtype=np.float32)
    Wo = np.asarray(Wo, dtype=np.float32)

    nc = _get_module()
    in_maps = _prep_in_maps(hs, mask, ms, Wq, Wk, Wv, Wo)

    kwargs = {}
    if TRACE:
        kwargs = {"trace": True}
        if TRACE_CORES is not None:
            kwargs["trace_cores"] = TRACE_CORES
    res = run_bass_kernel_spmd(nc, in_maps, core_ids=list(range(N_CORES)), **kwargs)
    LAST_RESULT = res

    yp = [r["ypT"] for r in res.results]  # each [SLOTS, HID] f32
    y = np.stack(
        [yp[GROUPS * b] + yp[GROUPS * b + 1] for b in range(B)], axis=0
    )
    return np.ascontiguousarray(y.astype(np.float32))


# revision 29
# speedup vs baseline: 1.1638x; 1.0057x over previous
"""Trainium2 Bass kernel: memory-slot cross-attention (nn_LocalConstructorMulti).

Reference computation (per batch b):
    Q  = memory_slots @ Wq.T                      [slots, BD]    (shared over b)
    K  = hs_b @ Wk.T                              [S, BD]
    V  = hs_b @ Wv.T                              [S, BD]
    s  = (Q_h . K_h) / sqrt(HD)  + mask           [heads, slots, S]
    p  = softmax(s, axis=S)
    o  = p @ V_h                                  [heads, slots, HD]
    y  = concat_h(o) @ Wo.T                       [slots, HID]

Sharding: 8 cores = 4 batches x 2 head-groups (4 heads / 256 bottleneck dims
each).  Each core sees the full hidden states of its batch and a 256-wide
slice of Wk/Wv/Wo, computes the full softmax locally over its heads, and
produces a partial y (contribution of its 4 heads).  The host sums the two
partials per batch -- o_proj is linear in the per-head outputs.

Key layout/engine decisions (v2, rebuilt from the TimelineSim bottleneck
analysis of v1: PE sequencer + HWDGE were saturated by 2434 small matmuls and
519 small-descriptor DMAs while the PE array itself was 45% idle):

  - hs is shipped twice, host-preshuffled into [8 blocks, 128 ki, 16 pair,
    2 two, 512 col] so each block is ONE DMA with 16-32 KiB contiguous
    per-partition lines: an fp8 copy (x8 scale) feeding the K-projection and
    a bf16 copy feeding the V-projection.  19 DMAs total vs 519.
  - K-projection runs in fp8 with MatmulPerfMode.DoubleRow: stationary
    wk8[128,2,128] x moving hs8[128,2,512] contracts TWO 128-deep k-subtiles
    per instruction (256 instructions for the whole KT).  fp8 noise on the
    K side is attenuated ~64x by the near-uniform softmax, contributing
    ~0.1% to the output.  Wk is pre-scaled x64 so its N(0, 1/4096) entries
    sit in e4m3's normal range; the combined 512x scale is folded into Q.
  - V-projection stays bf16 (V noise passes straight into the output):
    stationary hs blocks [128,128] x moving wv [128,256], PSUM-accumulated
    over all 32 k-subtiles, 1024 instructions at full 128x128x256 occupancy.
  - scores are built per 128-row tile with TWO head-pairs stacked on
    partitions: stationary kt[128, 128rows] x moving block-diagonal
    q2[128, 16] -> 64 matmuls; mask is a per-partition bias fused into Exp.
  - o = p^T @ V_aug runs as ONE 32-instruction PSUM chain: stationary
    pt[128, 4h*8n] x moving v[128, 4h*65] computes all 4 heads at once
    (the off-diagonal head blocks are discarded); the ones-column yields
    sum(p) for free, normalization is a per-partition scalar multiply.
  - o_proj is emitted transposed: yT[8, 4096] = ot[bd,8]^T @ woT[bd,4096] in
    16 mov-512 matmuls; the host adds the two head-group partials per batch.
  - Q (8x512, 0.02% of FLOPs) is computed on host and shipped pre-scaled as
    the block-diagonal q2 operand; every PSUM accumulator owns a full bank
    (PE-W + DVE/ACT-R same-bank erratum).
"""

import sys

if "/opt/trn_rl_repo" not in sys.path:
    sys.path.insert(0, "/opt/trn_rl_repo")

import ml_dtypes
import numpy as np

import concourse.bass as bass  # noqa: F401  (AP helpers)
import concourse.mybir as mybir
import concourse.tile as tile
from concourse import bacc
from concourse.bass_utils import run_bass_kernel_spmd
from concourse.masks import make_identity

BF16 = mybir.dt.bfloat16
FP8 = mybir.dt.float8e4
F32 = mybir.dt.float32
npbf16 = ml_dtypes.bfloat16
npfp8 = ml_dtypes.float8_e4m3

B, S, HID = 4, 4096, 4096
SLOTS, HEADS, BD = 8, 8, 512
HD = BD // HEADS  # 64
N_CORES = 8
GROUPS = N_CORES // B  # head-groups per batch
HPC = HEADS // GROUPS  # heads per core = 4
BDC = HPC * HD  # bottleneck slice per core = 256
MASK_NEG = -30000.0

# hs8 is cast on-device from the bf16 copy (unscaled: K-path noise is
# attenuated ~64x by the softmax, so e4m3 subnormal loss is irrelevant);
# all fp8 range scaling lives in Wk (N(0, 1/4096) entries x512 -> N(0, 8))
WK_SCALE = 512.0
# total score scale 1/sqrt(HD) divided back out of the device-side K product
Q_SCALE = 1.0 / (np.sqrt(HD) * WK_SCALE)

NBLK = 8  # column blocks of the sequence
CBLK = S // NBLK  # 512 columns per block
NPAIR = HID // 256  # 16 k-subtile pairs
NRT = S // 128  # 32 row tiles

# test.py can flip this to capture an NTFF profile; harness never touches it.
TRACE = False
TRACE_CORES = None
LAST_RESULT = None

_cache = {}


def _build_module():
    """Emit + compile the single-core Bass module (same NEFF on all cores)."""
    nc = bacc.Bacc("TRN2", target_bir_lowering=False, debug=False, num_devices=N_CORES)

    hsbT = nc.dram_tensor("hsbT", [NBLK, 128, NPAIR, 2, CBLK], BF16, kind="ExternalInput").ap()
    wk8T = nc.dram_tensor("wk8T", [128, NPAIR, 2, BDC], FP8, kind="ExternalInput").ap()
    wvT = nc.dram_tensor("wvT", [128, NPAIR, 2, BDC], BF16, kind="ExternalInput").ap()
    woT = nc.dram_tensor("woT", [128, 2, HID], BF16, kind="ExternalInput").ap()
    q2T = nc.dram_tensor("q2T", [128, 2, 2 * SLOTS], BF16, kind="ExternalInput").ap()
    mbT = nc.dram_tensor("mbT", [128, NRT], F32, kind="ExternalInput").ap()
    ypT = nc.dram_tensor("ypT", [SLOTS, HID], F32, kind="ExternalOutput").ap()

    DR = mybir.MatmulPerfMode.DoubleRow

    with tile.TileContext(nc) as tc:
        with (
            tc.tile_pool(name="consts", bufs=1) as consts,
            tc.tile_pool(name="hs8p", bufs=2) as hs8p,
            tc.tile_pool(name="hsbp", bufs=2) as hsbp,
        ):
            # ---- resident weights / tables -------------------------------
            wk8_sb = consts.tile([128, NPAIR, 2, BDC], FP8)
            wv_sb = consts.tile([128, NPAIR, 2, BDC], BF16)
            # wo/q2/mb are DMA'd after block 0's hs tiles (they're needed
            # only by the lagged attention / final o_proj)
            wo_sb = consts.tile([128, 2, HID], BF16)
            q2_sb = consts.tile([128, 2, 2 * SLOTS], BF16)
            mb_sb = consts.tile([128, NRT], F32)
            ident = consts.tile([128, 128], BF16)
            make_identity(nc, ident)

            # ---- persistent intermediates --------------------------------
            kt_sb = consts.tile([128, 2, S], BF16)  # K.T x512  [bd, rows]
            v_sb = consts.tile([128, NRT, HPC, HD + 1], BF16)  # V rows + ones
            nc.vector.memset(v_sb[:, :, :, HD : HD + 1], 1.0)
            # exp(scores).T, head stride padded to 32 so each head's o-block
            # lands on a 32-partition boundary (engine partition-offset rule)
            pt_sb = consts.tile([128, NRT, HPC, 32], BF16)
            nc.vector.memset(pt_sb, 0.0)

            # ---- K/V projections + lagged attention, one streamed pass ---
            # attention for block b-1 (scores -> exp -> o-chain partial) is
            # emitted after block b's projections, so the PE never waits on
            # the ACT exp round-trip and the old 15-20us serial tail folds
            # into the DMA/PE overlap window.
            RPB = CBLK // 128  # rowtiles per block

            def emit_attn(blk):
                for i in range(blk * RPB, (blk + 1) * RPB):
                    s_ps = sps.tile([128, 512], F32, tag="s")
                    for m2 in range(2):
                        nc.tensor.matmul(
                            s_ps[:, m2 * 16 : (m2 + 1) * 16],
                            kt_sb[:, m2, i * 128 : (i + 1) * 128],
                            q2_sb[:, m2, :],
                            start=True,
                            stop=True,
                        )
                    nc.scalar.activation(
                        out=pt_sb[:, i, :, 0:SLOTS],
                        in_=s_ps[:, 0 : HPC * SLOTS].rearrange(
                            "p (h n) -> p h n", h=HPC
                        ),
                        func=mybir.ActivationFunctionType.Exp,
                        bias=mb_sb[:, i : i + 1],
                        scale=1.0,
                    )
                for i in range(blk * RPB, (blk + 1) * RPB):
                    nc.tensor.matmul(
                        o_ps[:, 0 : HPC * (HD + 1)],
                        pt_sb[:, i, :, :],
                        v_sb[:, i, :, :],
                        start=(i == 0),
                        stop=(i == NRT - 1),
                    )

            with (
                tc.tile_pool(name="ktps", bufs=2, space="PSUM") as ktps,
                tc.tile_pool(name="vps", bufs=2, space="PSUM") as vps,
                tc.tile_pool(name="sps", bufs=2, space="PSUM") as sps,
                tc.tile_pool(name="ops", bufs=1, space="PSUM") as ops,
            ):
                o_ps = ops.tile([128, 512], F32)
                for blk in range(NBLK):
                    # hs DMAs arrive in pair-group chunks so the K/V chains
                    # start consuming before the whole block has landed;
                    # the V-path operands stream first (V is the long pole)
                    hsb_t = hsbp.tile([128, NPAIR, 2, CBLK], BF16, tag="hsb")
                    hs8_t = hs8p.tile([128, NPAIR, 2, CBLK], FP8, tag="hs8")
                    cast_engines = (
                        nc.vector.tensor_copy,
                        nc.scalar.copy,
                    )
                    for q in range(8):
                        h = NPAIR // 8
                        if blk == 0:
                            # wv pair-chunks ride along with the first hsb
                            # chunks: the V chain consumes pairs in order
                            nc.sync.dma_start(
                                out=wv_sb[:, q * h : (q + 1) * h],
                                in_=wvT[:, q * h : (q + 1) * h],
                            )
                        nc.sync.dma_start(
                            out=hsb_t[:, q * h : (q + 1) * h],
                            in_=hsbT[blk][:, q * h : (q + 1) * h],
                        )
                        # hs8 is derived on-device: a casting copy on the
                        # otherwise-idle Pool/DVE/ACT engines replaces 16 MB
                        # of fp8 DMA traffic
                        cast_engines[q % 2](
                            out=hs8_t[:, q * h : (q + 1) * h],
                            in_=hsb_t[:, q * h : (q + 1) * h],
                        )
                    if blk == 0:
                        nc.sync.dma_start(out=wk8_sb, in_=wk8T)
                    if blk == 0:
                        nc.sync.dma_start(out=q2_sb, in_=q2T)
                        nc.sync.dma_start(out=mb_sb, in_=mbT)
                    if blk == NBLK - 1:
                        # wo is first needed by o_proj at the very end; keep
                        # it out of the hs stream so it causes no PE bubble
                        nc.sync.dma_start(out=wo_sb, in_=woT)
                    # V chains: bf16, stationary hs blocks, full-bank psum
                    for r in range(RPB):
                        v_ps = vps.tile([128, 512], F32, tag="v")
                        for p in range(NPAIR):
                            for t in range(2):
                                nc.tensor.matmul(
                                    v_ps[:, 0:BDC],
                                    hsb_t[:, p, t, r * 128 : (r + 1) * 128],
                                    wv_sb[:, p, t, :],
                                    start=(p == 0 and t == 0),
                                    stop=(p == NPAIR - 1 and t == 1),
                                )
                        rt = blk * RPB + r
                        nc.vector.tensor_copy(
                            out=v_sb[:, rt, :, 0:HD],
                            in_=v_ps[:, 0:BDC].rearrange("p (h d) -> p h d", h=HPC),
                        )
                    # KT chains: fp8 DoubleRow, 2 k-subtiles per instruction
                    for m2 in range(2):
                        kt_ps = ktps.tile([128, CBLK], F32, tag="kt")
                        for p in range(NPAIR):
                            nc.tensor.matmul(
                                kt_ps,
                                wk8_sb[:, p, :, m2 * 128 : (m2 + 1) * 128],
                                hs8_t[:, p, :, :],
                                start=(p == 0),
                                stop=(p == NPAIR - 1),
                                perf_mode=DR,
                            )
                        nc.scalar.copy(
                            out=kt_sb[:, m2, blk * CBLK : (blk + 1) * CBLK],
                            in_=kt_ps,
                        )
                    emit_attn(blk)
                # normalize straight out of PSUM: o / sum(p) via the ones
                # column (the o-chain has stopped, so no same-bank PE-W
                # overlaps these reads)
                recip = consts.tile([SLOTS, HPC], F32)
                o_slot = consts.tile([SLOTS, BDC], BF16)
                for h in range(HPC):
                    nc.vector.reciprocal(
                        out=recip[:, h : h + 1],
                        in_=o_ps[h * 32 : h * 32 + SLOTS,
                                 h * (HD + 1) + HD : (h + 1) * (HD + 1)],
                    )
                for h in range(HPC):
                    nc.vector.tensor_scalar_mul(
                        out=o_slot[:, h * HD : (h + 1) * HD],
                        in0=o_ps[h * 32 : h * 32 + SLOTS,
                                 h * (HD + 1) : h * (HD + 1) + HD],
                        scalar1=recip[:, h : h + 1],
                    )

            # ---- transpose o to [bd, slots] ------------------------------
            ot_sb = consts.tile([128, 2, SLOTS], BF16)
            with tc.tile_pool(name="tps", bufs=2, space="PSUM") as tps:
                for j in range(2):
                    t_ps = tps.tile([128, 1024], BF16, tag="t")
                    nc.tensor.transpose(
                        t_ps[:, 0:SLOTS],
                        o_slot[:, j * 128 : (j + 1) * 128],
                        ident[:SLOTS, :SLOTS],
                    )
                    nc.scalar.copy(out=ot_sb[:, j, :], in_=t_ps[:, 0:SLOTS])

            # ---- partial o_proj, transposed: yT = ot^T @ woT -------------
            # drains alternate ACT/DVE so neither engine paces the phase;
            # each 512-seg is DMA'd out as soon as its copy lands
            y_sb = consts.tile([SLOTS, HID], F32)
            with tc.tile_pool(name="yps", bufs=4, space="PSUM") as yps:
                for seg in range(HID // 512):
                    y_ps = yps.tile([SLOTS, 512], F32, tag="y")
                    for j in range(2):
                        nc.tensor.matmul(
                            y_ps,
                            ot_sb[:, j, :],
                            wo_sb[:, j, seg * 512 : (seg + 1) * 512],
                            start=(j == 0),
                            stop=(j == 1),
                        )
                    if seg % 2 == 0:
                        nc.scalar.copy(
                            out=y_sb[:, seg * 512 : (seg + 1) * 512], in_=y_ps
                        )
                    else:
                        nc.vector.tensor_copy(
                            out=y_sb[:, seg * 512 : (seg + 1) * 512], in_=y_ps
                        )
                    if seg % 2 == 1:
                        nc.sync.dma_start(
                            out=ypT[:, (seg - 1) * 512 : (seg + 1) * 512],
                            in_=y_sb[:, (seg - 1) * 512 : (seg + 1) * 512],
                        )

    nc.compile()
    return nc


def _get_module():
    if "m" not in _cache:
        _cache["m"] = _build_module()
    return _cache["m"]


def _shuffle_hs(hsT_np, dtype, scale=1.0):
    """[HID, S] -> [NBLK, 128, NPAIR, 2, CBLK] with the (pair, two, ki)
    k-decomposition on axis 0 and (blk, col) on axis 1."""
    a = hsT_np.reshape(NPAIR, 2, 128, NBLK, CBLK)
    a = a.transpose(3, 2, 0, 1, 4)  # blk, ki, pair, two, col
    if scale != 1.0:
        a = a * np.float32(scale)
    return np.ascontiguousarray(a.astype(dtype))


def _prep_in_maps(hs, mask, ms, Wq, Wk, Wv, Wo):
    """Shard the full inputs into 8 per-core input maps (host-side)."""
    WkT = Wk.T.astype(np.float32)  # [HID, BD]
    WvT = Wv.T.astype(np.float32)
    WoT = Wo.T.astype(np.float32)  # [BD, HID]
    Q = (ms @ Wq.T).astype(np.float32)  # [SLOTS, BD]

    hsb = []
    mb = []
    for b in range(B):
        hsT = np.ascontiguousarray(hs[b].T)  # [HID, S]
        hsb.append(_shuffle_hs(hsT, npbf16))
        mb.append(
            np.ascontiguousarray(
                np.where(mask[b] == 0, np.float32(MASK_NEG), np.float32(0.0))
                .astype(np.float32)
                .reshape(NRT, 128)
                .T
            )
        )

    in_maps = []
    for c in range(N_CORES):
        b, g = c // GROUPS, c % GROUPS
        sl = slice(g * BDC, (g + 1) * BDC)
        wk8 = (
            (WkT[:, sl] * np.float32(WK_SCALE))
            .reshape(NPAIR, 2, 128, BDC)
            .transpose(2, 0, 1, 3)
        )
        wv = WvT[:, sl].reshape(NPAIR, 2, 128, BDC).transpose(2, 0, 1, 3)
        wo = WoT[sl].reshape(2, 128, HID).transpose(1, 0, 2)
        q2 = np.zeros((128, 2, 2 * SLOTS), np.float32)
        for m2 in range(2):
            h0 = g * HPC + 2 * m2
            q2[0:64, m2, 0:SLOTS] = Q[:, h0 * HD : (h0 + 1) * HD].T * Q_SCALE
            q2[64:128, m2, SLOTS : 2 * SLOTS] = (
                Q[:, (h0 + 1) * HD : (h0 + 2) * HD].T * Q_SCALE
            )
        in_maps.append(
            {
                "hsbT": hsb[b],
                "wk8T": np.ascontiguousarray(wk8.astype(npfp8)),
                "wvT": np.ascontiguousarray(wv.astype(npbf16)),
                "woT": np.ascontiguousarray(wo.astype(npbf16)),
                "q2T": np.ascontiguousarray(q2.astype(npbf16)),
                "mbT": mb[b],
            }
        )
    return in_maps


def kernel(hidden_states, attention_mask, memory_slots, Wq, Wk, Wv, Wo):
    global LAST_RESULT
    hs = np.asarray(hidden_states, dtype=np.float32)
    mask = np.asarray(attention_mask)
    ms = np.asarray(memory_slots, dtype=np.float32)
    Wq = np.asarray(Wq, dtype=np.float32)
    Wk = np.asarray(Wk, dtype=np.float32)
    Wv = np.asarray(Wv, dtype=np.float32)
    Wo = np.asarray(Wo, dtype=np.float32)

    nc = _get_module()
    in_maps = _prep_in_maps(hs, mask, ms, Wq, Wk, Wv, Wo)

    kwargs = {}
    if TRACE:
        kwargs = {"trace": True}
        if TRACE_CORES is not None:
            kwargs["trace_cores"] = TRACE_CORES
    res = run_bass_kernel_spmd(nc, in_maps, core_ids=list(range(N_CORES)), **kwargs)
    LAST_RESULT = res

    yp = [r["ypT"] for r in res.results]  # each [SLOTS, HID] f32
    y = np.stack(
        [yp[GROUPS * b] + yp[GROUPS * b + 1] for b in range(B)], axis=0
    )
    return np.ascontiguousarray(y.astype(np.float32))


# revision 33
# speedup vs baseline: 1.1790x; 1.0131x over previous
"""Trainium2 Bass kernel: memory-slot cross-attention (nn_LocalConstructorMulti).

Reference computation (per batch b):
    Q  = memory_slots @ Wq.T                      [slots, BD]    (shared over b)
    K  = hs_b @ Wk.T                              [S, BD]
    V  = hs_b @ Wv.T                              [S, BD]
    s  = (Q_h . K_h) / sqrt(HD)  + mask           [heads, slots, S]
    p  = softmax(s, axis=S)
    o  = p @ V_h                                  [heads, slots, HD]
    y  = concat_h(o) @ Wo.T                       [slots, HID]

Sharding: 8 cores = 4 batches x 2 head-groups (4 heads / 256 bottleneck dims
each).  Each core sees the full hidden states of its batch and a 256-wide
slice of Wk/Wv/Wo, computes the full softmax locally over its heads, and
produces a partial y (contribution of its 4 heads).  The host sums the two
partials per batch -- o_proj is linear in the per-head outputs.

Key layout/engine decisions (v2, rebuilt from the TimelineSim bottleneck
analysis of v1: PE sequencer + HWDGE were saturated by 2434 small matmuls and
519 small-descriptor DMAs while the PE array itself was 45% idle):

  - hs is shipped twice, host-preshuffled into [8 blocks, 128 ki, 16 pair,
    2 two, 512 col] so each block is ONE DMA with 16-32 KiB contiguous
    per-partition lines: an fp8 copy (x8 scale) feeding the K-projection and
    a bf16 copy feeding the V-projection.  19 DMAs total vs 519.
  - K-projection runs in fp8 with MatmulPerfMode.DoubleRow: stationary
    wk8[128,2,128] x moving hs8[128,2,512] contracts TWO 128-deep k-subtiles
    per instruction (256 instructions for the whole KT).  fp8 noise on the
    K side is attenuated ~64x by the near-uniform softmax, contributing
    ~0.1% to the output.  Wk is pre-scaled x64 so its N(0, 1/4096) entries
    sit in e4m3's normal range; the combined 512x scale is folded into Q.
  - V-projection stays bf16 (V noise passes straight into the output):
    stationary hs blocks [128,128] x moving wv [128,256], PSUM-accumulated
    over all 32 k-subtiles, 1024 instructions at full 128x128x256 occupancy.
  - scores are built per 128-row tile with TWO head-pairs stacked on
    partitions: stationary kt[128, 128rows] x moving block-diagonal
    q2[128, 16] -> 64 matmuls; mask is a per-partition bias fused into Exp.
  - o = p^T @ V_aug runs as ONE 32-instruction PSUM chain: stationary
    pt[128, 4h*8n] x moving v[128, 4h*65] computes all 4 heads at once
    (the off-diagonal head blocks are discarded); the ones-column yields
    sum(p) for free, normalization is a per-partition scalar multiply.
  - o_proj is emitted transposed: yT[8, 4096] = ot[bd,8]^T @ woT[bd,4096] in
    16 mov-512 matmuls; the host adds the two head-group partials per batch.
  - Q (8x512, 0.02% of FLOPs) is computed on host and shipped pre-scaled as
    the block-diagonal q2 operand; every PSUM accumulator owns a full bank
    (PE-W + DVE/ACT-R same-bank erratum).
"""

import sys

if "/opt/trn_rl_repo" not in sys.path:
    sys.path.insert(0, "/opt/trn_rl_repo")

import ml_dtypes
import numpy as np

import concourse.bass as bass  # noqa: F401  (AP helpers)
import concourse.mybir as mybir
import concourse.tile as tile
from concourse import bacc
from concourse.bass_utils import run_bass_kernel_spmd
from concourse.masks import make_identity

BF16 = mybir.dt.bfloat16
FP8 = mybir.dt.float8e4
F32 = mybir.dt.float32
npbf16 = ml_dtypes.bfloat16
npfp8 = ml_dtypes.float8_e4m3

B, S, HID = 4, 4096, 4096
SLOTS, HEADS, BD = 8, 8, 512
HD = BD // HEADS  # 64
N_CORES = 8
GROUPS = N_CORES // B  # head-groups per batch
HPC = HEADS // GROUPS  # heads per core = 4
BDC = HPC * HD  # bottleneck slice per core = 256
MASK_NEG = -30000.0

# hs8 is cast on-device from the bf16 copy (unscaled: K-path noise is
# attenuated ~64x by the softmax, so e4m3 subnormal loss is irrelevant);
# all fp8 range scaling lives in Wk (N(0, 1/4096) entries x512 -> N(0, 8))
WK_SCALE = 512.0
# total score scale 1/sqrt(HD) divided back out of the device-side K product
Q_SCALE = 1.0 / (np.sqrt(HD) * WK_SCALE)

NBLK = 8  # column blocks of the sequence
CBLK = S // NBLK  # 512 columns per block
NPAIR = HID // 256  # 16 k-subtile pairs
NRT = S // 128  # 32 row tiles

# test.py can flip this to capture an NTFF profile; harness never touches it.
TRACE = False
TRACE_CORES = None
LAST_RESULT = None

_cache = {}


def _build_module():
    """Emit + compile the single-core Bass module (same NEFF on all cores)."""
    nc = bacc.Bacc("TRN2", target_bir_lowering=False, debug=False, num_devices=N_CORES)

    hsbT = nc.dram_tensor("hsbT", [NBLK, 128, NPAIR, 2, CBLK], BF16, kind="ExternalInput").ap()
    wk8T = nc.dram_tensor("wk8T", [128, NPAIR, 2, BDC], FP8, kind="ExternalInput").ap()
    wvT = nc.dram_tensor("wvT", [128, NPAIR, 2, BDC], BF16, kind="ExternalInput").ap()
    woT = nc.dram_tensor("woT", [128, 2, HID], BF16, kind="ExternalInput").ap()
    q2T = nc.dram_tensor("q2T", [128, 2, 2 * SLOTS], BF16, kind="ExternalInput").ap()
    mbT = nc.dram_tensor("mbT", [128, NRT], F32, kind="ExternalInput").ap()
    ypT = nc.dram_tensor("ypT", [SLOTS, HID], F32, kind="ExternalOutput").ap()

    DR = mybir.MatmulPerfMode.DoubleRow

    with tile.TileContext(nc) as tc:
        with (
            tc.tile_pool(name="consts", bufs=1) as consts,
            tc.tile_pool(name="hs8p", bufs=2) as hs8p,
            tc.tile_pool(name="hsbp", bufs=2) as hsbp,
        ):
            # ---- resident weights / tables -------------------------------
            wk8_sb = consts.tile([128, NPAIR, 2, BDC], FP8)
            wv_sb = consts.tile([128, NPAIR, 2, BDC], BF16)
            # wo/q2/mb are DMA'd after block 0's hs tiles (they're needed
            # only by the lagged attention / final o_proj)
            wo_sb = consts.tile([128, 2, HID], BF16)
            q2_sb = consts.tile([128, 2, 2 * SLOTS], BF16)
            mb_sb = consts.tile([128, NRT], F32)
            ident = consts.tile([128, 128], BF16)
            make_identity(nc, ident)

            # ---- persistent intermediates --------------------------------
            kt_sb = consts.tile([128, 2, S], BF16)  # K.T x512  [bd, rows]
            v_sb = consts.tile([128, NRT, HPC, HD + 1], BF16)  # V rows + ones
            nc.vector.memset(v_sb[:, :, :, HD : HD + 1], 1.0)
            # exp(scores).T, head stride padded to 32 so each head's o-block
            # lands on a 32-partition boundary (engine partition-offset rule)
            pt_sb = consts.tile([128, NRT, HPC, 32], BF16)
            nc.vector.memset(pt_sb, 0.0)

            # ---- K/V projections + lagged attention, one streamed pass ---
            # attention for block b-1 (scores -> exp -> o-chain partial) is
            # emitted after block b's projections, so the PE never waits on
            # the ACT exp round-trip and the old 15-20us serial tail folds
            # into the DMA/PE overlap window.
            RPB = CBLK // 128  # rowtiles per block

            def emit_attn(rt0, nrt_sub):
                for i in range(rt0, rt0 + nrt_sub):
                    s_ps = sps.tile([128, 512], F32, tag="s")
                    for m2 in range(2):
                        nc.tensor.matmul(
                            s_ps[:, m2 * 16 : (m2 + 1) * 16],
                            kt_sb[:, m2, i * 128 : (i + 1) * 128],
                            q2_sb[:, m2, :],
                            start=True,
                            stop=True,
                        )
                    nc.scalar.activation(
                        out=pt_sb[:, i, :, 0:SLOTS],
                        in_=s_ps[:, 0 : HPC * SLOTS].rearrange(
                            "p (h n) -> p h n", h=HPC
                        ),
                        func=mybir.ActivationFunctionType.Exp,
                        bias=mb_sb[:, i : i + 1],
                        scale=1.0,
                    )
                for i in range(rt0, rt0 + nrt_sub):
                    nc.tensor.matmul(
                        o_ps[:, 0 : HPC * (HD + 1)],
                        pt_sb[:, i, :, :],
                        v_sb[:, i, :, :],
                        start=(i == 0),
                        stop=(i == NRT - 1),
                    )

            with (
                tc.tile_pool(name="ktps", bufs=2, space="PSUM") as ktps,
                tc.tile_pool(name="vps", bufs=2, space="PSUM") as vps,
                tc.tile_pool(name="sps", bufs=2, space="PSUM") as sps,
                tc.tile_pool(name="ops", bufs=1, space="PSUM") as ops,
            ):
                o_ps = ops.tile([128, 512], F32)
                cast_engines = (
                    nc.vector.tensor_copy,
                    nc.scalar.copy,
                )
                # block 0 is processed as two 256-col halves so the first V
                # chain starts after ~1/4 of a block has streamed instead of
                # a full one (PE fill); the rest are full 512-col blocks
                SUBS = [(0, 0, CBLK // 2), (0, CBLK // 2, CBLK // 2)] + [
                    (b, 0, CBLK) for b in range(1, NBLK)
                ]
                for sub, (blk, c0, w) in enumerate(SUBS):
                    # hs DMAs arrive in pair-group chunks so the K/V chains
                    # start consuming before the whole sub-block has landed
                    hsb_t = hsbp.tile([128, NPAIR, 2, CBLK], BF16, tag="hsb")
                    hs8_t = hs8p.tile([128, NPAIR, 2, CBLK], FP8, tag="hs8")
                    nch = 8 * w // CBLK
                    h = NPAIR // nch
                    for q in range(nch):
                        if sub == 0:
                            # wv pair-chunks ride along with the first hsb
                            # chunks: the V chain consumes pairs in order
                            nc.sync.dma_start(
                                out=wv_sb[:, q * h : (q + 1) * h],
                                in_=wvT[:, q * h : (q + 1) * h],
                            )
                        nc.sync.dma_start(
                            out=hsb_t[:, q * h : (q + 1) * h, :, 0:w],
                            in_=hsbT[blk][:, q * h : (q + 1) * h, :, c0 : c0 + w],
                        )
                        # hs8 is derived on-device: a casting copy on the
                        # otherwise-idle DVE/ACT engines replaces 16 MB of
                        # fp8 DMA traffic
                        cast_engines[q % 2](
                            out=hs8_t[:, q * h : (q + 1) * h, :, 0:w],
                            in_=hsb_t[:, q * h : (q + 1) * h, :, 0:w],
                        )
                    if sub == 0:
                        nc.sync.dma_start(out=wk8_sb, in_=wk8T)
                        nc.sync.dma_start(out=q2_sb, in_=q2T)
                        nc.sync.dma_start(out=mb_sb, in_=mbT)
                    if sub == len(SUBS) - 1:
                        # wo is first needed by o_proj at the very end; keep
                        # it out of the hs stream so it causes no PE bubble
                        nc.sync.dma_start(out=wo_sb, in_=woT)
                    # V chains: bf16, stationary hs blocks, full-bank psum
                    for r in range(w // 128):
                        v_ps = vps.tile([128, 512], F32, tag="v")
                        for p in range(NPAIR):
                            for t in range(2):
                                nc.tensor.matmul(
                                    v_ps[:, 0:BDC],
                                    hsb_t[:, p, t, r * 128 : (r + 1) * 128],
                                    wv_sb[:, p, t, :],
                                    start=(p == 0 and t == 0),
                                    stop=(p == NPAIR - 1 and t == 1),
                                )
                        rt = (blk * CBLK + c0) // 128 + r
                        nc.vector.tensor_copy(
                            out=v_sb[:, rt, :, 0:HD],
                            in_=v_ps[:, 0:BDC].rearrange("p (h d) -> p h d", h=HPC),
                        )
                    # KT chains: fp8 DoubleRow, 2 k-subtiles per instruction
                    for m2 in range(2):
                        kt_ps = ktps.tile([128, CBLK], F32, tag="kt")
                        for p in range(NPAIR):
                            nc.tensor.matmul(
                                kt_ps[:, 0:w],
                                wk8_sb[:, p, :, m2 * 128 : (m2 + 1) * 128],
                                hs8_t[:, p, :, 0:w],
                                start=(p == 0),
                                stop=(p == NPAIR - 1),
                                perf_mode=DR,
                            )
                        nc.scalar.copy(
                            out=kt_sb[
                                :, m2, blk * CBLK + c0 : blk * CBLK + c0 + w
                            ],
                            in_=kt_ps[:, 0:w],
                        )
                    emit_attn((blk * CBLK + c0) // 128, w // 128)
                # normalize straight out of PSUM: o / sum(p) via the ones
                # column (the o-chain has stopped, so no same-bank PE-W
                # overlaps these reads)
                recip = consts.tile([SLOTS, HPC], F32)
                o_slot = consts.tile([SLOTS, BDC], BF16)
                for h in range(HPC):
                    nc.vector.reciprocal(
                        out=recip[:, h : h + 1],
                        in_=o_ps[h * 32 : h * 32 + SLOTS,
                                 h * (HD + 1) + HD : (h + 1) * (HD + 1)],
                    )
                for h in range(HPC):
                    nc.vector.tensor_scalar_mul(
                        out=o_slot[:, h * HD : (h + 1) * HD],
                        in0=o_ps[h * 32 : h * 32 + SLOTS,
                                 h * (HD + 1) : h * (HD + 1) + HD],
                        scalar1=recip[:, h : h + 1],
                    )

            # ---- transpose o to [bd, slots] ------------------------------
            ot_sb = consts.tile([128, 2, SLOTS], BF16)
            with tc.tile_pool(name="tps", bufs=2, space="PSUM") as tps:
                for j in range(2):
                    t_ps = tps.tile([128, 1024], BF16, tag="t")
                    nc.tensor.transpose(
                        t_ps[:, 0:SLOTS],
                        o_slot[:, j * 128 : (j + 1) * 128],
                        ident[:SLOTS, :SLOTS],
                    )
                    nc.scalar.copy(out=ot_sb[:, j, :], in_=t_ps[:, 0:SLOTS])

            # ---- partial o_proj, transposed: yT = ot^T @ woT -------------
            # drains alternate ACT/DVE so neither engine paces the phase;
            # each 512-seg is DMA'd out as soon as its copy lands
            y_sb = consts.tile([SLOTS, HID], F32)
            with tc.tile_pool(name="yps", bufs=4, space="PSUM") as yps:
                for seg in range(HID // 512):
                    y_ps = yps.tile([SLOTS, 512], F32, tag="y")
                    for j in range(2):
                        nc.tensor.matmul(
                            y_ps,
                            ot_sb[:, j, :],
                            wo_sb[:, j, seg * 512 : (seg + 1) * 512],
                            start=(j == 0),
                            stop=(j == 1),
                        )
                    if seg % 2 == 0:
                        nc.scalar.copy(
                            out=y_sb[:, seg * 512 : (seg + 1) * 512], in_=y_ps
                        )
                    else:
                        nc.vector.tensor_copy(
                            out=y_sb[:, seg * 512 : (seg + 1) * 512], in_=y_ps
                        )
                    if seg % 2 == 1:
                        nc.sync.dma_start(
                            out=ypT[:, (seg - 1) * 512 : (seg + 1) * 512],
                            in_=y_sb[:, (seg - 1) * 512 : (seg + 1) * 512],
                        )

    nc.compile()
    return nc


def _get_module():
    if "m" not in _cache:
        _cache["m"] = _build_module()
    return _cache["m"]


def _shuffle_hs(hsT_np, dtype, scale=1.0):
    """[HID, S] -> [NBLK, 128, NPAIR, 2, CBLK] with the (pair, two, ki)
    k-decomposition on axis 0 and (blk, col) on axis 1."""
    a = hsT_np.reshape(NPAIR, 2, 128, NBLK, CBLK)
    a = a.transpose(3, 2, 0, 1, 4)  # blk, ki, pair, two, col
    if scale != 1.0:
        a = a * np.float32(scale)
    return np.ascontiguousarray(a.astype(dtype))


def _prep_in_maps(hs, mask, ms, Wq, Wk, Wv, Wo):
    """Shard the full inputs into 8 per-core input maps (host-side)."""
    WkT = Wk.T.astype(np.float32)  # [HID, BD]
    WvT = Wv.T.astype(np.float32)
    WoT = Wo.T.astype(np.float32)  # [BD, HID]
    Q = (ms @ Wq.T).astype(np.float32)  # [SLOTS, BD]

    hsb = []
    mb = []
    for b in range(B):
        hsT = np.ascontiguousarray(hs[b].T)  # [HID, S]
        hsb.append(_shuffle_hs(hsT, npbf16))
        mb.append(
            np.ascontiguousarray(
                np.where(mask[b] == 0, np.float32(MASK_NEG), np.float32(0.0))
                .astype(np.float32)
                .reshape(NRT, 128)
                .T
            )
        )

    in_maps = []
    for c in range(N_CORES):
        b, g = c // GROUPS, c % GROUPS
        sl = slice(g * BDC, (g + 1) * BDC)
        wk8 = (
            (WkT[:, sl] * np.float32(WK_SCALE))
            .reshape(NPAIR, 2, 128, BDC)
            .transpose(2, 0, 1, 3)
        )
        wv = WvT[:, sl].reshape(NPAIR, 2, 128, BDC).transpose(2, 0, 1, 3)
        wo = WoT[sl].reshape(2, 128, HID).transpose(1, 0, 2)
        q2 = np.zeros((128, 2, 2 * SLOTS), np.float32)
        for m2 in range(2):
            h0 = g * HPC + 2 * m2
            q2[0:64, m2, 0:SLOTS] = Q[:, h0 * HD : (h0 + 1) * HD].T * Q_SCALE
            q2[64:128, m2, SLOTS : 2 * SLOTS] = (
                Q[:, (h0 + 1) * HD : (h0 + 2) * HD].T * Q_SCALE
            )
        in_maps.append(
            {
                "hsbT": hsb[b],
                "wk8T": np.ascontiguousarray(wk8.astype(npfp8)),
                "wvT": np.ascontiguousarray(wv.astype(npbf16)),
                "woT": np.ascontiguousarray(wo.astype(npbf16)),
                "q2T": np.ascontiguousarray(q2.astype(npbf16)),
                "mbT": mb[b],
            }
        )
    return in_maps


def kernel(hidden_states, attention_mask, memory_slots, Wq, Wk, Wv, Wo):
    global LAST_RESULT
    hs = np.asarray(hidden_states, dtype=np.float32)
    mask = np.asarray(attention_mask)
    ms = np.asarray(memory_slots, dtype=np.float32)
    Wq = np.asarray(Wq, dtype=np.float32)
    Wk = np.asarray(Wk, dtype=np.float32)
    Wv = np.asarray(Wv, dtype=np.float32)
    Wo = np.asarray(Wo, dtype=np.float32)

    nc = _get_module()
    in_maps = _prep_in_maps(hs, mask, ms, Wq, Wk, Wv, Wo)

    kwargs = {}
    if TRACE:
        kwargs = {"trace": True}
        if TRACE_CORES is not None:
            kwargs["trace_cores"] = TRACE_CORES
    res = run_bass_kernel_spmd(nc, in_maps, core_ids=list(range(N_CORES)), **kwargs)
    LAST_RESULT = res

    yp = [r["ypT"] for r in res.results]  # each [SLOTS, HID] f32
    y = np.stack(
        [yp[GROUPS * b] + yp[GROUPS * b + 1] for b in range(B)], axis=0
    )
    return np.ascontiguousarray(y.astype(np.float32))
